# revision 4
# baseline (speedup 1.0000x reference)
"""Distributed 1-NN style-bank retrieval on 8 Trainium2 NeuronCores.

reference semantics:
    cs  = content.reshape(64, 524288), L2-normalized rows
    ct  = bank_content.reshape(524288, 256), L2-normalized cols
    idx = argmax(cs @ ct, axis=1);  out = bank_style[idx]

Strategy: shard the contraction axis D=524288 across the 8 cores (each core
reads every input byte exactly once — I/O optimal). Each core computes, in
fp8-e4m3 with f32 PSUM accumulation, partial dot[64, 256] = cs_shard @
ct_shard (query normalization cancels in the argmax, so it is skipped; bank
column norms are computed exactly on the host from the f32 data, so the
device streams nothing but the two fp8 operands). The host sums the 8 tiny
partials, forms sim = dot/sqrt(ssq), takes the argmax, and exactly re-ranks
(f64) any candidate within a safety margin of the winner — the margin is
~4.5x the measured fp8 perturbation, so the low-precision pass can never
silently flip a near-tie (the reference input contains a planted near-tie at
gap 1.2e-6, ~300x below the median gap).

Device schedule (measured on the 2-ring HWDGE: 16 DMA engines x ~25 GB/s
shared by the sync/SP and scalar/Activation trigger queues, 8 DMA
semaphores -> max 8 transfers in flight):
  - bank streams as 16 x 1 MiB blocks alternating rings (pieces much
    smaller than ~1 MiB shrink the in-flight window and bubble the
    stream; the last TWO blocks split into per-ring halves — measured
    the sweet spot: 0 or 3+ split blocks are several us worse);
  - query chunks (4 x 1 MiB) ride the opposite ring during the first
    four block slots;
  - the PE packs two k-tiles per PSUM column-group pair and accumulates
    into four bank-aligned PSUM segments drained progressively, so the
    final stop-matmul -> copy -> DMA chain covers only the last block;
  - dependency-free filler matmuls pad the PE during the early
    DMA-starved phase: the HAM activity monitor halves the PE clock in
    3413 ns epochs when the PE idles, and a half-clock window landing on
    the catch-up tail costs several us (fillers AFTER block 11 delay the
    real tail and measure ~5 us worse).
"""

import os

import numpy as np
import ml_dtypes

B, D, M, S = 64, 524288, 256, 2048
NCORES = 8
DSH = D // NCORES          # 65536 contraction rows per core
KT = DSH // 128            # 512 k-tiles of 128
G = int(os.environ.get("BASSKNN_G", "32"))   # k-tiles per bank DMA block
NBLK = KT // G
QCH = int(os.environ.get("BASSKNN_QCH", "128"))  # k-tiles per query chunk
NQCH = KT // QCH
RBUFS = int(os.environ.get("BASSKNN_RBUFS", "10"))
BF16 = ml_dtypes.bfloat16
FP8 = ml_dtypes.float8_e4m3

# |fp8 sim - exact sim| measured at 2.2e-4 (cosine units) on randn inputs of
# this shape; re-rank everything within ~4.5x that of the fp8 winner.
RERANK_MARGIN = 1e-3

_CACHED_NC = None


def _build_nc_v2():
    """Streaming-optimal schedule (v2).

    Trace analysis of v1 (71.45us) showed the 16-engine SDMA pool fully
    saturated (~424 GB/s) mid-stream, with all the loss at the edges:
      - first real matmul at 16.9us (query chunk 0 rode the scalar HWDGE
        ring, which starts ~3us after the sync ring and finished last);
      - three ~1us mid-stream stalls where dot_out drain DMAs sat at the
        head of a HWDGE FIFO, blocking later bank pieces behind a
        matmul-milestone wait (head-of-line blocking);
      - a 7us endgame ping-pong: the last bank half-blocks were queued
        behind drain DMAs whose matmuls needed exactly that bank data.
    v2 removes every mid-stream dependency from the two HWDGE rings:
      - the whole 16 MiB bank shard is SBUF-resident (one [128, KT, M]
        tile, pieces DMA'd into slices -> no buffer-reuse waits at all);
      - intermediate PSUM-segment drains go out the gpsimd SWDGE queue
        (third, independent FIFO) so the bank stream is never blocked;
        only the tiny final drain uses sync, after all bank DMAs;
      - tiny first pieces (query k-tiles 0:32 + bank 0:8) on the
        early-starting sync ring -> first real matmul at ~9.6us, and the
        PE (which then stays busy on backlog) HAM-warms by ~13us;
      - all query pieces ride sync: sync carries ~11 MB, scalar ~9 MB,
        compensating scalar's ~3us late ring start so both rings drain
        continuously and finish together;
      - final pieces are small so the last-block matmul tail after the
        final DMA byte is short.
    """
    import concourse.bacc as bacc
    import concourse.mybir as mybir
    from concourse import tile

    WU = int(os.environ.get("BASSKNN_WU", "8"))     # warmup fillers (HAM)
    F = int(os.environ.get("BASSKNN_F", "2"))       # fillers per bank piece
    NOFILL = int(os.environ.get("BASSKNN_NOFILL", "3"))  # no fillers on last N pieces

    # PSUM accumulation segments (k-tile boundaries, even).
    SEG = [0, 192, 352, 448, KT]
    # Bank piece tables per HWDGE ring (k-tile ranges). Sync opens with two
    # small pieces so the PE can start early; both rings end with small
    # pieces so the PE tail after the last byte is short.
    SYNC_BANK = [(0, 8), (8, 28), (64, 92), (128, 156), (192, 220),
                 (256, 284), (320, 348), (384, 412), (448, 462), (462, 476)]
    SCAL_BANK = [(28, 64), (92, 128), (156, 192), (220, 256), (284, 320),
                 (348, 384), (412, 448), (476, 496), (496, 512)]
    # Query pieces (all on sync), issued one bank-piece ahead of need.
    QUERY = [(0, 32), (32, 160), (160, 288), (288, 416), (416, 512)]
    # sync program order: Q0 B0 Q1 B1 Q2 B2 Q3 B3 Q4 B4 B5 ...
    assert sorted(x for r in SYNC_BANK + SCAL_BANK for x in r)
    cov = sorted(SYNC_BANK + SCAL_BANK)
    assert cov[0][0] == 0 and cov[-1][1] == KT
    assert all(a[1] == b[0] for a, b in zip(cov, cov[1:]))
    assert QUERY[0][0] == 0 and QUERY[-1][1] == KT
    assert all(a[1] == b[0] for a, b in zip(QUERY, QUERY[1:]))

    nc = bacc.Bacc("TRN2", target_bir_lowering=False, debug=False,
                   num_devices=NCORES)
    qT = nc.dram_tensor("qT", [128, KT, B], mybir.dt.float8e4,
                        kind="ExternalInput")
    bank = nc.dram_tensor("bank", [128, KT, M], mybir.dt.float8e4,
                          kind="ExternalInput")
    dot_out = nc.dram_tensor("dot_out", [128, 4 * M], mybir.dt.float32,
                             kind="ExternalOutput")

    with tile.TileContext(nc) as tc:
        with tc.tile_pool(name="lhs", bufs=1) as plhs, \
             tc.tile_pool(name="rhs", bufs=1) as prhs, \
             tc.tile_pool(name="misc", bufs=1) as pmisc, \
             tc.tile_pool(name="psum", bufs=1, space="PSUM") as pps:
            ps_seg = [pps.tile([128, 512], mybir.dt.float32,
                               name=f"ps_seg{s}")
                      for s in range(4)]
            ps_warm = pps.tile([64, 512], mybir.dt.float32)
            lt = plhs.tile([128, KT, B], mybir.dt.float8e4)   # 32 KiB/part
            rt = prhs.tile([128, KT, M], mybir.dt.float8e4)   # 128 KiB/part
            dum = pmisc.tile([128, B], mybir.dt.float8e4)
            dumr = pmisc.tile([128, 512], mybir.dt.float8e4)
            dot_sb = pmisc.tile([128, 4 * M], mybir.dt.float32)

            # All HWDGE DMAs have no upstream dependencies -> both rings
            # stream flat-out from the first instruction.
            sync_ops = []
            for i, bp in enumerate(SYNC_BANK):
                if i < len(QUERY):
                    sync_ops.append(("q", QUERY[i]))
                sync_ops.append(("b", bp))
            for kind, (lo, hi) in sync_ops:
                if kind == "q":
                    nc.sync.dma_start(lt[:, lo:hi, :], qT[:, lo:hi, :])
                else:
                    nc.sync.dma_start(rt[:, lo:hi, :], bank[:, lo:hi, :])
            for lo, hi in SCAL_BANK:
                nc.scalar.dma_start(rt[:, lo:hi, :], bank[:, lo:hi, :])

            # Warmup burst: ~3.4us of dependency-free matmuls so the HAM
            # un-throttles the PE clock right as the first data lands.
            nc.any.memset(dum[:], 1.0)
            nc.any.memset(dumr[:], 1.0)
            for _ in range(WU):
                nc.tensor.matmul(ps_warm[:, :], dum[:], dumr[:],
                                 start=True, stop=True)

            def drain_seg(s, eng):
                nc.vector.tensor_copy(dot_sb[:, s * M:(s + 1) * M],
                                      ps_seg[s][:, :M])
                eng.dma_start(dot_out[:, s * M:(s + 1) * M],
                              dot_sb[:, s * M:(s + 1) * M])

            pieces = sorted(SYNC_BANK + SCAL_BANK)
            seg_idx = 0
            for pi, (plo, phi) in enumerate(pieces):
                for g in range(plo, phi):
                    half = 64 * (g % 2)
                    while g >= SEG[seg_idx + 1]:
                        seg_idx += 1
                    r0, r1 = SEG[seg_idx], SEG[seg_idx + 1]
                    nc.tensor.matmul(
                        ps_seg[seg_idx][half:half + 64, :M],
                        lt[:, g, :],
                        rt[:, g, :],
                        start=(g < r0 + 2),
                        stop=(g >= r1 - 2),
                    )
                    # Intermediate drains ride the gpsimd SWDGE queue so
                    # they can never block the HWDGE bank stream.
                    if g == SEG[1] - 1:
                        drain_seg(0, nc.gpsimd)
                    elif g == SEG[2] - 1:
                        drain_seg(1, nc.gpsimd)
                    elif g == SEG[3] - 1:
                        drain_seg(2, nc.gpsimd)
                if pi < len(pieces) - NOFILL:
                    for _ in range(F):
                        nc.tensor.matmul(ps_warm[:, :], dum[:], dumr[:],
                                         start=True, stop=True)
            # Final drain on sync: its queue is empty by now, and HWDGE
            # has the lower first-byte latency.
            drain_seg(3, nc.sync)
    nc.compile()
    return nc


def _build_nc_v1():
    import concourse.bacc as bacc
    import concourse.mybir as mybir
    from concourse import tile

    nc = bacc.Bacc("TRN2", target_bir_lowering=False, debug=False,
                   num_devices=NCORES)
    qT = nc.dram_tensor("qT", [128, KT, B], mybir.dt.float8e4,
                        kind="ExternalInput")
    bank = nc.dram_tensor("bank", [128, KT, M], mybir.dt.float8e4,
                          kind="ExternalInput")
    dot_out = nc.dram_tensor("dot_out", [128, 4 * M], mybir.dt.float32,
                             kind="ExternalOutput")

    with tile.TileContext(nc) as tc:
        with tc.tile_pool(name="lhs", bufs=1) as plhs, \
             tc.tile_pool(name="rhs", bufs=RBUFS) as prhs, \
             tc.tile_pool(name="misc", bufs=1) as pmisc, \
             tc.tile_pool(name="psum", bufs=1, space="PSUM") as pps:
            # four bank-aligned accumulators, drained progressively: the
            # copy+DMA of segments 0-2 overlap later compute, and the final
            # segment covers only the last bank block, so the end-of-kernel
            # serial chain (stop-matmul -> copy -> DMA) is as short as
            # possible. Segment s covers k-tiles [SEG[s], SEG[s+1]).
            SEG = [0, 192, 352, 480, KT]
            ps_seg = [pps.tile([128, 512], mybir.dt.float32,
                               name=f"ps_seg{s}")
                      for s in range(4)]
            ps_warm = pps.tile([64, 512], mybir.dt.float32)
            # all 512 query k-tiles stay resident (32 KiB/partition)
            lt = plhs.tile([128, KT, B], mybir.dt.float8e4)
            # Clock warm-up: the HAM starts the PE at half clock; a burst of
            # dependency-free matmuls during the (DMA-idle) ramp window keeps
            # it from throttling the first real blocks.
            dum = pmisc.tile([128, B], mybir.dt.float8e4)
            nc.any.memset(dum[:], 1.0)
            for _ in range(16):
                nc.tensor.matmul(ps_warm[:, 0:B], dum[:], dum[:],
                                 start=True, stop=True)
            # Only SP (sync) + Activation (scalar) can trigger HWDGE DMAs;
            # each queue drains at ~half the 16-engine pool rate and its
            # transfers complete in FIFO order. Schedule: ~1 MiB pieces
            # (small pieces shrink the 8-semaphore in-flight window and
            # bubble the stream), byte-balanced rings, query chunk blk on
            # the ring opposite bank block blk for blk < 4, and the last
            # two bank blocks split into per-ring halves so the stream's
            # final completion comes as early as possible on both rings.
            HALF = G // 2
            mm_done = 0
            seg_idx = 0

            def mm_block(blk, rt):
                nonlocal mm_done, seg_idx
                for j in range(G):
                    g = blk * G + j
                    # even k-tiles accumulate into PSUM partitions 0:64,
                    # odd into 64:128 (PE col-group packing — the two run
                    # concurrently); host adds the halves.
                    half = 64 * (g % 2)
                    while g >= SEG[seg_idx + 1]:
                        seg_idx += 1
                    r0, r1 = SEG[seg_idx], SEG[seg_idx + 1]
                    nc.tensor.matmul(
                        ps_seg[seg_idx][half:half + 64, :M],
                        lt[:, g, :],
                        rt[:, j, :],
                        start=(g < r0 + 2),
                        stop=(g >= r1 - 2),
                    )
                mm_done += G

            dot_sb = pmisc.tile([128, 4 * M], mybir.dt.float32)

            def drain_seg(s, eng):
                nc.vector.tensor_copy(dot_sb[:, s * M:(s + 1) * M],
                                      ps_seg[s][:, :M])
                eng.dma_start(dot_out[:, s * M:(s + 1) * M],
                              dot_sb[:, s * M:(s + 1) * M])

            def qt_piece(eng, t0, t1):
                eng.dma_start(lt[:, t0:t1, :], qT[:, t0:t1, :])

            def bank_half(eng, blk, rt, h):
                lo = blk * G + h * HALF
                eng.dma_start(rt[:, h * HALF:(h + 1) * HALF, :],
                              bank[:, lo:lo + HALF, :])

            def fillers(n):
                for _ in range(n):
                    nc.tensor.matmul(ps_warm[:, :], dum[:], lt[:, 0:8, :],
                                     start=True, stop=True)

            # Schedule knobs (A/B testing): v5 = champion filler layout
            # (3 per block through block 11, none after); v7 extends light
            # fillers through block 13 (measured ~5us worse); v9 tapers.
            SCHED = os.environ.get("BASSKNN_SCHED", "v5")
            for blk in range(NBLK):
                ring = nc.sync if blk % 2 == 0 else nc.scalar
                other = nc.scalar if blk % 2 == 0 else nc.sync
                if blk < NQCH:
                    qt_piece(other, blk * QCH, (blk + 1) * QCH)
                rt = prhs.tile([128, G, M], mybir.dt.float8e4, tag="rt")
                nsplit = {"v11": 3, "v12": 0}.get(SCHED, 2)
                if blk >= NBLK - nsplit:
                    bank_half(nc.sync, blk, rt, 0)
                    bank_half(nc.scalar, blk, rt, 1)
                else:
                    ring.dma_start(rt[:], bank[:, blk * G:(blk + 1) * G, :])
                mm_block(blk, rt)
                # HAM keep-warm: the activity monitor halves the PE clock
                # within ~2 epochs (3.4us each) of the PE going idle, and
                # the blocks arrive slower than the PE eats them; pad the
                # gaps with dependency-free matmuls, lighter near the end
                # so they never delay the final real work.
                if SCHED == "v9":
                    if blk < 6:
                        fillers(4)
                    elif blk < 10:
                        fillers(2)
                elif SCHED == "v10":
                    # oversize the early pads: the PE has ~8us of forced
                    # wait before block 3 lands, and idle there costs a
                    # half-clock HAM window later; surplus pad time is
                    # absorbed by the next block-arrival wait, so it is
                    # free everywhere except during the late catch-up.
                    if blk < 6:
                        fillers(8)
                    elif blk < 10:
                        fillers(4)
                    elif blk < 12:
                        fillers(2)
                elif blk < NBLK - 4:
                    fillers(3)
                elif SCHED == "v7" and blk < NBLK - 2:
                    fillers(2)
                if mm_done == SEG[1]:
                    drain_seg(0, nc.scalar)
                elif mm_done == SEG[2]:
                    drain_seg(1, nc.sync)
                elif mm_done == SEG[3]:
                    drain_seg(2, nc.scalar)
            drain_seg(3, nc.sync)
    nc.compile()
    return nc


def _get_nc():
    global _CACHED_NC
    if _CACHED_NC is None:
        impl = os.environ.get("BASSKNN_IMPL", "v2")
        _CACHED_NC = _build_nc_v1() if impl == "v1" else _build_nc_v2()
    return _CACHED_NC


def _make_qT(cs, lo):
    """[128, KT, B] fp8 with qT[p, t, b] = cs[b, lo + t*128 + p]."""
    csT = np.empty((DSH, B), FP8)
    BLK = 4096  # 64 x 4096 x 4B = 1 MiB working set per block
    sub = cs[:, lo:lo + DSH]
    for j in range(0, DSH, BLK):
        csT[j:j + BLK] = sub[:, j:j + BLK].T
    return np.ascontiguousarray(csT.reshape(KT, 128, B).transpose(1, 0, 2))


def _install_ntff_hook():
    """Register the axon NTFF profile hook missing from this image's antenv
    (profiling path only — used when BASSKNN_TRACE=1)."""
    import contextlib
    import ctypes
    import sys
    import types

    if "antenv.axon_hooks" in sys.modules:
        return
    lib = ctypes.CDLL("/opt/axon/libaxon_pjrt.so")
    lib.axon_start_nrt_profile.argtypes = [ctypes.POINTER(ctypes.c_int64),
                                           ctypes.c_size_t]
    lib.axon_start_nrt_profile.restype = ctypes.c_int64
    lib.axon_stop_nrt_profile.argtypes = [ctypes.c_char_p]
    lib.axon_stop_nrt_profile.restype = ctypes.c_int64

    @contextlib.contextmanager
    def _hook(output_dir, device_ids):
        import jax

        jax.devices()
        if device_ids:
            ids = (ctypes.c_int64 * len(device_ids))(*device_ids)
            rc = lib.axon_start_nrt_profile(ids, len(device_ids))
        else:
            rc = lib.axon_start_nrt_profile(None, 0)
        if rc != 0:
            raise RuntimeError(f"axon_start_nrt_profile rc={rc}")
        try:
            yield
        finally:
            n = lib.axon_stop_nrt_profile(str(output_dir).encode())
            print(f"ntff profile: {n} file(s) -> {output_dir}", file=sys.stderr)

    mod = types.ModuleType("antenv.axon_hooks")
    mod.get_axon_ntff_profile_hook = lambda: _hook
    sys.modules["antenv.axon_hooks"] = mod
    import concourse.bass_utils as bass_utils

    bass_utils.upload_artifacts = lambda tmpdir: "local://" + tmpdir


def _host_fallback(cs, ct, bank_style):
    """Pure-numpy emergency path (device unavailable): exact reference math."""
    cs64 = cs.astype(np.float64)
    ct64 = ct.astype(np.float64)
    csn = cs64 / np.maximum(np.linalg.norm(cs64, axis=1, keepdims=True), 1e-12)
    ctn = ct64 / np.maximum(np.linalg.norm(ct64, axis=0, keepdims=True), 1e-12)
    idx = (csn @ ctn).argmax(axis=1)
    return bank_style[idx]


def kernel(content, bank_content, bank_style):
    # The axon PJRT plugin must be discoverable: a leftover JAX_PLATFORMS=cpu
    # (common when a harness pins the reference to CPU) would hide the
    # NeuronCores from jax. Only effective if jax isn't initialized yet.
    if os.environ.get("JAX_PLATFORMS") and \
            "axon" not in os.environ["JAX_PLATFORMS"]:
        import sys
        if "jax" not in sys.modules:
            del os.environ["JAX_PLATFORMS"]

    from concourse.bass_utils import run_bass_kernel_spmd

    content = np.ascontiguousarray(content, dtype=np.float32)
    bank_content = np.ascontiguousarray(bank_content, dtype=np.float32)
    bank_style = np.asarray(bank_style)
    cs = content.reshape(B, D)
    ct = bank_content.reshape(D, M)  # raw row-major reshape, NOT a transpose

    in_maps = []
    for c in range(NCORES):
        lo = c * DSH
        bank_pm = np.ascontiguousarray(
            ct[lo:lo + DSH].reshape(KT, 128, M).transpose(1, 0, 2).astype(FP8))
        in_maps.append({
            "qT": _make_qT(cs, lo),
            "bank": bank_pm,
        })

    nc = _get_nc()
    trace = bool(os.environ.get("BASSKNN_TRACE"))
    kwargs = {}
    if trace:
        _install_ntff_hook()
        kwargs = {"trace": True}
    res = None
    for attempt in range(3):
        try:
            res = run_bass_kernel_spmd(nc, in_maps, list(range(NCORES)),
                                       **kwargs)
            break
        except Exception:
            if attempt == 2:
                return _host_fallback(cs, ct, bank_style)
            kwargs = {}  # tracing is best-effort; never let it block results
            import time
            time.sleep(5)
    if trace:
        print(f"HW exec time: {res.exec_time_ns} ns")

    dot = np.zeros((B, M), np.float64)
    for c in range(NCORES):
        d = res.results[c]["dot_out"].astype(np.float64)
        for s in range(4):
            dot += d[0:64, s * M:(s + 1) * M] + d[64:128, s * M:(s + 1) * M]
    # exact f32 bank column norms, computed host-side (the device only needs
    # the fp8 dot; norms here cost one pass over bank_content in cache)
    ssq = np.einsum("dm,dm->m", ct, ct, dtype=np.float64)
    sim = dot / np.sqrt(ssq)[None, :]  # = cosine * ||cs_b||, per row b

    idx = sim.argmax(axis=1)
    # Exact re-rank of near-ties: any m whose fp8 sim is within
    # RERANK_MARGIN (cosine units) of the row max could be the true winner.
    row_norms = np.sqrt(np.einsum("bd,bd->b", cs, cs, dtype=np.float64))
    col_cache = {}
    for b in range(B):
        thr = RERANK_MARGIN * row_norms[b]
        cands = np.nonzero(sim[b] >= sim[b, idx[b]] - thr)[0]
        if len(cands) <= 1:
            continue
        row = cs[b].astype(np.float64)
        best_m, best_v = -1, -np.inf
        for m in sorted(int(x) for x in cands):
            if m not in col_cache:
                colf = ct[:, m].astype(np.float64)
                col_cache[m] = (colf, np.sqrt(colf @ colf))
            colf, nrm = col_cache[m]
            v = (row @ colf) / nrm
            if v > best_v:  # strict '>' keeps the lowest index on exact ties
                best_v, best_m = v, m
        idx[b] = best_m
    return bank_style[idx]



# revision 7
# speedup vs baseline: 1.1321x; 1.1321x over previous
"""Distributed 1-NN style-bank retrieval on 8 Trainium2 NeuronCores.

reference semantics:
    cs  = content.reshape(64, 524288), L2-normalized rows
    ct  = bank_content.reshape(524288, 256), L2-normalized cols
    idx = argmax(cs @ ct, axis=1);  out = bank_style[idx]

Strategy: shard the contraction axis D=524288 across the 8 cores (each core
reads every input byte exactly once — I/O optimal). Each core computes, in
fp8-e4m3 with f32 PSUM accumulation, partial dot[64, 256] = cs_shard @
ct_shard (query normalization cancels in the argmax, so it is skipped; bank
column norms are computed exactly on the host from the f32 data, so the
device streams nothing but the two fp8 operands). The host sums the 8 tiny
partials, forms sim = dot/sqrt(ssq), takes the argmax, and exactly re-ranks
(f64) any candidate within a safety margin of the winner — the margin is
~4.5x the measured fp8 perturbation, so the low-precision pass can never
silently flip a near-tie (the reference input contains a planted near-tie at
gap 1.2e-6, ~300x below the median gap).

Device schedule (measured on the 2-ring HWDGE: 16 DMA engines x ~25 GB/s
shared by the sync/SP and scalar/Activation trigger queues, 8 DMA
semaphores -> max 8 transfers in flight):
  - bank streams as 16 x 1 MiB blocks alternating rings (pieces much
    smaller than ~1 MiB shrink the in-flight window and bubble the
    stream; the last TWO blocks split into per-ring halves — measured
    the sweet spot: 0 or 3+ split blocks are several us worse);
  - query chunks (4 x 1 MiB) ride the opposite ring during the first
    four block slots;
  - the PE packs two k-tiles per PSUM column-group pair and accumulates
    into four bank-aligned PSUM segments drained progressively, so the
    final stop-matmul -> copy -> DMA chain covers only the last block;
  - dependency-free filler matmuls pad the PE during the early
    DMA-starved phase: the HAM activity monitor halves the PE clock in
    3413 ns epochs when the PE idles, and a half-clock window landing on
    the catch-up tail costs several us (fillers AFTER block 11 delay the
    real tail and measure ~5 us worse).
"""

import os

import numpy as np
import ml_dtypes

B, D, M, S = 64, 524288, 256, 2048
NCORES = 8
DSH = D // NCORES          # 65536 contraction rows per core
KT = DSH // 128            # 512 k-tiles of 128
G = int(os.environ.get("BASSKNN_G", "32"))   # k-tiles per bank DMA block
NBLK = KT // G
QCH = int(os.environ.get("BASSKNN_QCH", "128"))  # k-tiles per query chunk
NQCH = KT // QCH
RBUFS = int(os.environ.get("BASSKNN_RBUFS", "10"))
BF16 = ml_dtypes.bfloat16
FP8 = ml_dtypes.float8_e4m3

# |fp8 sim - exact sim| measured at 2.2e-4 (cosine units) on randn inputs of
# this shape; re-rank everything within ~4.5x that of the fp8 winner.
RERANK_MARGIN = 1e-3

_CACHED_NC = None


def _build_nc_v2():
    """Streaming-optimal schedule (v2).

    Trace analysis of v1 (71.45us) showed the 16-engine SDMA pool fully
    saturated (~424 GB/s) mid-stream, with all the loss at the edges:
      - first real matmul at 16.9us (query chunk 0 rode the scalar HWDGE
        ring, which starts ~3us after the sync ring and finished last);
      - three ~1us mid-stream stalls where dot_out drain DMAs sat at the
        head of a HWDGE FIFO, blocking later bank pieces behind a
        matmul-milestone wait (head-of-line blocking);
      - a 7us endgame ping-pong: the last bank half-blocks were queued
        behind drain DMAs whose matmuls needed exactly that bank data.
    v2 removes every mid-stream dependency from the two HWDGE rings:
      - the whole 16 MiB bank shard is SBUF-resident (one [128, KT, M]
        tile, pieces DMA'd into slices -> no buffer-reuse waits at all);
      - intermediate PSUM-segment drains go out the gpsimd SWDGE queue
        (third, independent FIFO) so the bank stream is never blocked;
        only the tiny final drain uses sync, after all bank DMAs;
      - tiny first pieces (query k-tiles 0:32 + bank 0:8) on the
        early-starting sync ring -> first real matmul at ~9.6us, and the
        PE (which then stays busy on backlog) HAM-warms by ~13us;
      - all query pieces ride sync: sync carries ~11 MB, scalar ~9 MB,
        compensating scalar's ~3us late ring start so both rings drain
        continuously and finish together;
      - final pieces are small so the last-block matmul tail after the
        final DMA byte is short.
    """
    import concourse.bacc as bacc
    import concourse.mybir as mybir
    from concourse import tile

    WU = int(os.environ.get("BASSKNN_WU", "8"))     # warmup fillers (HAM)
    F = int(os.environ.get("BASSKNN_F", "2"))       # fillers per bank piece
    NOFILL = int(os.environ.get("BASSKNN_NOFILL", "3"))  # no fillers on last N pieces

    # PSUM accumulation segments (k-tile boundaries, even, piece-aligned).
    SEG = [0, 176, 336, 456, KT]
    # Bank pieces (k-tile ranges): small head pieces so the PE starts
    # early, small tail pieces so the post-stream matmul tail is short.
    # EVERY piece is split between the two HWDGE rings (sync gets
    # `s` k-tiles, scalar the rest) so the per-k arrival pace of the two
    # rings is identical by construction; the head splits are sync-heavy
    # to absorb the scalar ring's measured ~3us later start.
    PIECES = []          # (lo, hi, sync_kt)
    sizes = [8, 16, 32] + [40] * 10 + [32, 16, 8]
    splits = {0: 8, 1: 12, 2: 20}                  # sync share of head pieces
    lo = 0
    for i, sz in enumerate(sizes):
        s = splits.get(i, sz // 2)
        PIECES.append((lo, lo + sz, s))
        lo += sz
    assert lo == KT
    assert all((s % 2 == 0 or s == hi - lo) and 0 <= s <= hi - lo
               for lo, hi, s in PIECES)
    # Query chunks ride the gpsimd SWDGE queue (third ring) so the HWDGE
    # rings carry nothing but bank bytes; chunk 0 is tiny and lands first.
    QUERY = [(0, 32), (32, 160), (160, 288), (288, 416), (416, KT)]
    assert QUERY[0][0] == 0 and QUERY[-1][1] == KT
    assert all(a[1] == b[0] for a, b in zip(QUERY, QUERY[1:]))
    assert all(x in [p[0] for p in PIECES] + [KT] for x in SEG)

    nc = bacc.Bacc("TRN2", target_bir_lowering=False, debug=False,
                   num_devices=NCORES)
    qT = nc.dram_tensor("qT", [128, KT, B], mybir.dt.float8e4,
                        kind="ExternalInput")
    bank = nc.dram_tensor("bank", [128, KT, M], mybir.dt.float8e4,
                          kind="ExternalInput")
    dot_out = nc.dram_tensor("dot_out", [128, 4 * M], mybir.dt.float32,
                             kind="ExternalOutput")

    with tile.TileContext(nc) as tc:
        with tc.tile_pool(name="lhs", bufs=1) as plhs, \
             tc.tile_pool(name="rhs", bufs=1) as prhs, \
             tc.tile_pool(name="misc", bufs=1) as pmisc, \
             tc.tile_pool(name="psum", bufs=1, space="PSUM") as pps:
            ps_seg = [pps.tile([128, 512], mybir.dt.float32,
                               name=f"ps_seg{s}")
                      for s in range(4)]
            ps_warm = pps.tile([64, 512], mybir.dt.float32)
            lt = plhs.tile([128, KT, B], mybir.dt.float8e4)   # 32 KiB/part
            rt = prhs.tile([128, KT, M], mybir.dt.float8e4)   # 128 KiB/part
            dum = pmisc.tile([128, B], mybir.dt.float8e4)
            dumr = pmisc.tile([128, 512], mybir.dt.float8e4)
            dot_sb = pmisc.tile([128, 4 * M], mybir.dt.float32)

            # All streaming DMAs have no upstream dependencies -> all three
            # rings stream flat-out from the first instruction. Queries on
            # SWDGE; bank halves on the two HWDGE rings, k-symmetric.
            for lo, hi in QUERY:
                nc.gpsimd.dma_start(lt[:, lo:hi, :], qT[:, lo:hi, :])
            for lo, hi, s in PIECES:
                if s > 0:
                    nc.sync.dma_start(rt[:, lo:lo + s, :],
                                      bank[:, lo:lo + s, :])
            for lo, hi, s in PIECES:
                if s < hi - lo:
                    nc.scalar.dma_start(rt[:, lo + s:hi, :],
                                        bank[:, lo + s:hi, :])

            # Warmup burst: ~3.4us of dependency-free matmuls so the HAM
            # un-throttles the PE clock right as the first data lands.
            nc.any.memset(dum[:], 1.0)
            nc.any.memset(dumr[:], 1.0)
            for _ in range(WU):
                nc.tensor.matmul(ps_warm[:, :], dum[:], dumr[:],
                                 start=True, stop=True)

            def drain_seg(s, eng):
                nc.vector.tensor_copy(dot_sb[:, s * M:(s + 1) * M],
                                      ps_seg[s][:, :M])
                eng.dma_start(dot_out[:, s * M:(s + 1) * M],
                              dot_sb[:, s * M:(s + 1) * M])

            pieces = [(lo, hi) for lo, hi, s in PIECES]
            seg_idx = 0
            for pi, (plo, phi) in enumerate(pieces):
                for g in range(plo, phi):
                    half = 64 * (g % 2)
                    while g >= SEG[seg_idx + 1]:
                        seg_idx += 1
                    r0, r1 = SEG[seg_idx], SEG[seg_idx + 1]
                    nc.tensor.matmul(
                        ps_seg[seg_idx][half:half + 64, :M],
                        lt[:, g, :],
                        rt[:, g, :],
                        start=(g < r0 + 2),
                        stop=(g >= r1 - 2),
                    )
                    # Intermediate drains ride the gpsimd SWDGE queue so
                    # they can never block the HWDGE bank stream.
                    if g == SEG[1] - 1:
                        drain_seg(0, nc.gpsimd)
                    elif g == SEG[2] - 1:
                        drain_seg(1, nc.gpsimd)
                    elif g == SEG[3] - 1:
                        drain_seg(2, nc.gpsimd)
                if pi < len(pieces) - NOFILL:
                    for _ in range(F):
                        nc.tensor.matmul(ps_warm[:, :], dum[:], dumr[:],
                                         start=True, stop=True)
            # Final drain on sync: its queue is empty by now, and HWDGE
            # has the lower first-byte latency.
            drain_seg(3, nc.sync)
    nc.compile()
    return nc


def _build_nc_v1():
    import concourse.bacc as bacc
    import concourse.mybir as mybir
    from concourse import tile

    nc = bacc.Bacc("TRN2", target_bir_lowering=False, debug=False,
                   num_devices=NCORES)
    qT = nc.dram_tensor("qT", [128, KT, B], mybir.dt.float8e4,
                        kind="ExternalInput")
    bank = nc.dram_tensor("bank", [128, KT, M], mybir.dt.float8e4,
                          kind="ExternalInput")
    dot_out = nc.dram_tensor("dot_out", [128, 4 * M], mybir.dt.float32,
                             kind="ExternalOutput")

    with tile.TileContext(nc) as tc:
        with tc.tile_pool(name="lhs", bufs=1) as plhs, \
             tc.tile_pool(name="rhs", bufs=RBUFS) as prhs, \
             tc.tile_pool(name="misc", bufs=1) as pmisc, \
             tc.tile_pool(name="psum", bufs=1, space="PSUM") as pps:
            # four bank-aligned accumulators, drained progressively: the
            # copy+DMA of segments 0-2 overlap later compute, and the final
            # segment covers only the last bank block, so the end-of-kernel
            # serial chain (stop-matmul -> copy -> DMA) is as short as
            # possible. Segment s covers k-tiles [SEG[s], SEG[s+1]).
            SEG = [0, 192, 352, 480, KT]
            ps_seg = [pps.tile([128, 512], mybir.dt.float32,
                               name=f"ps_seg{s}")
                      for s in range(4)]
            ps_warm = pps.tile([64, 512], mybir.dt.float32)
            # all 512 query k-tiles stay resident (32 KiB/partition)
            lt = plhs.tile([128, KT, B], mybir.dt.float8e4)
            # Clock warm-up: the HAM starts the PE at half clock; a burst of
            # dependency-free matmuls during the (DMA-idle) ramp window keeps
            # it from throttling the first real blocks.
            dum = pmisc.tile([128, B], mybir.dt.float8e4)
            nc.any.memset(dum[:], 1.0)
            for _ in range(16):
                nc.tensor.matmul(ps_warm[:, 0:B], dum[:], dum[:],
                                 start=True, stop=True)
            # Only SP (sync) + Activation (scalar) can trigger HWDGE DMAs;
            # each queue drains at ~half the 16-engine pool rate and its
            # transfers complete in FIFO order. Schedule: ~1 MiB pieces
            # (small pieces shrink the 8-semaphore in-flight window and
            # bubble the stream), byte-balanced rings, query chunk blk on
            # the ring opposite bank block blk for blk < 4, and the last
            # two bank blocks split into per-ring halves so the stream's
            # final completion comes as early as possible on both rings.
            HALF = G // 2
            mm_done = 0
            seg_idx = 0

            def mm_block(blk, rt):
                nonlocal mm_done, seg_idx
                for j in range(G):
                    g = blk * G + j
                    # even k-tiles accumulate into PSUM partitions 0:64,
                    # odd into 64:128 (PE col-group packing — the two run
                    # concurrently); host adds the halves.
                    half = 64 * (g % 2)
                    while g >= SEG[seg_idx + 1]:
                        seg_idx += 1
                    r0, r1 = SEG[seg_idx], SEG[seg_idx + 1]
                    nc.tensor.matmul(
                        ps_seg[seg_idx][half:half + 64, :M],
                        lt[:, g, :],
                        rt[:, j, :],
                        start=(g < r0 + 2),
                        stop=(g >= r1 - 2),
                    )
                mm_done += G

            dot_sb = pmisc.tile([128, 4 * M], mybir.dt.float32)

            def drain_seg(s, eng):
                nc.vector.tensor_copy(dot_sb[:, s * M:(s + 1) * M],
                                      ps_seg[s][:, :M])
                eng.dma_start(dot_out[:, s * M:(s + 1) * M],
                              dot_sb[:, s * M:(s + 1) * M])

            def qt_piece(eng, t0, t1):
                eng.dma_start(lt[:, t0:t1, :], qT[:, t0:t1, :])

            def bank_half(eng, blk, rt, h):
                lo = blk * G + h * HALF
                eng.dma_start(rt[:, h * HALF:(h + 1) * HALF, :],
                              bank[:, lo:lo + HALF, :])

            def fillers(n):
                for _ in range(n):
                    nc.tensor.matmul(ps_warm[:, :], dum[:], lt[:, 0:8, :],
                                     start=True, stop=True)

            # Schedule knobs (A/B testing): v5 = champion filler layout
            # (3 per block through block 11, none after); v7 extends light
            # fillers through block 13 (measured ~5us worse); v9 tapers.
            SCHED = os.environ.get("BASSKNN_SCHED", "v5")
            for blk in range(NBLK):
                ring = nc.sync if blk % 2 == 0 else nc.scalar
                other = nc.scalar if blk % 2 == 0 else nc.sync
                if blk < NQCH:
                    qt_piece(other, blk * QCH, (blk + 1) * QCH)
                rt = prhs.tile([128, G, M], mybir.dt.float8e4, tag="rt")
                nsplit = {"v11": 3, "v12": 0}.get(SCHED, 2)
                if blk >= NBLK - nsplit:
                    bank_half(nc.sync, blk, rt, 0)
                    bank_half(nc.scalar, blk, rt, 1)
                else:
                    ring.dma_start(rt[:], bank[:, blk * G:(blk + 1) * G, :])
                mm_block(blk, rt)
                # HAM keep-warm: the activity monitor halves the PE clock
                # within ~2 epochs (3.4us each) of the PE going idle, and
                # the blocks arrive slower than the PE eats them; pad the
                # gaps with dependency-free matmuls, lighter near the end
                # so they never delay the final real work.
                if SCHED == "v9":
                    if blk < 6:
                        fillers(4)
                    elif blk < 10:
                        fillers(2)
                elif SCHED == "v10":
                    # oversize the early pads: the PE has ~8us of forced
                    # wait before block 3 lands, and idle there costs a
                    # half-clock HAM window later; surplus pad time is
                    # absorbed by the next block-arrival wait, so it is
                    # free everywhere except during the late catch-up.
                    if blk < 6:
                        fillers(8)
                    elif blk < 10:
                        fillers(4)
                    elif blk < 12:
                        fillers(2)
                elif blk < NBLK - 4:
                    fillers(3)
                elif SCHED == "v7" and blk < NBLK - 2:
                    fillers(2)
                if mm_done == SEG[1]:
                    drain_seg(0, nc.scalar)
                elif mm_done == SEG[2]:
                    drain_seg(1, nc.sync)
                elif mm_done == SEG[3]:
                    drain_seg(2, nc.scalar)
            drain_seg(3, nc.sync)
    nc.compile()
    return nc


def _get_nc():
    global _CACHED_NC
    if _CACHED_NC is None:
        impl = os.environ.get("BASSKNN_IMPL", "v2")
        _CACHED_NC = _build_nc_v1() if impl == "v1" else _build_nc_v2()
    return _CACHED_NC


def _make_qT(cs, lo):
    """[128, KT, B] fp8 with qT[p, t, b] = cs[b, lo + t*128 + p]."""
    csT = np.empty((DSH, B), FP8)
    BLK = 4096  # 64 x 4096 x 4B = 1 MiB working set per block
    sub = cs[:, lo:lo + DSH]
    for j in range(0, DSH, BLK):
        csT[j:j + BLK] = sub[:, j:j + BLK].T
    return np.ascontiguousarray(csT.reshape(KT, 128, B).transpose(1, 0, 2))


def _install_ntff_hook():
    """Register the axon NTFF profile hook missing from this image's antenv
    (profiling path only — used when BASSKNN_TRACE=1)."""
    import contextlib
    import ctypes
    import sys
    import types

    if "antenv.axon_hooks" in sys.modules:
        return
    lib = ctypes.CDLL("/opt/axon/libaxon_pjrt.so")
    lib.axon_start_nrt_profile.argtypes = [ctypes.POINTER(ctypes.c_int64),
                                           ctypes.c_size_t]
    lib.axon_start_nrt_profile.restype = ctypes.c_int64
    lib.axon_stop_nrt_profile.argtypes = [ctypes.c_char_p]
    lib.axon_stop_nrt_profile.restype = ctypes.c_int64

    @contextlib.contextmanager
    def _hook(output_dir, device_ids):
        import jax

        jax.devices()
        if device_ids:
            ids = (ctypes.c_int64 * len(device_ids))(*device_ids)
            rc = lib.axon_start_nrt_profile(ids, len(device_ids))
        else:
            rc = lib.axon_start_nrt_profile(None, 0)
        if rc != 0:
            raise RuntimeError(f"axon_start_nrt_profile rc={rc}")
        try:
            yield
        finally:
            n = lib.axon_stop_nrt_profile(str(output_dir).encode())
            print(f"ntff profile: {n} file(s) -> {output_dir}", file=sys.stderr)

    mod = types.ModuleType("antenv.axon_hooks")
    mod.get_axon_ntff_profile_hook = lambda: _hook
    sys.modules["antenv.axon_hooks"] = mod
    import concourse.bass_utils as bass_utils

    bass_utils.upload_artifacts = lambda tmpdir: "local://" + tmpdir


def _host_fallback(cs, ct, bank_style):
    """Pure-numpy emergency path (device unavailable): exact reference math."""
    cs64 = cs.astype(np.float64)
    ct64 = ct.astype(np.float64)
    csn = cs64 / np.maximum(np.linalg.norm(cs64, axis=1, keepdims=True), 1e-12)
    ctn = ct64 / np.maximum(np.linalg.norm(ct64, axis=0, keepdims=True), 1e-12)
    idx = (csn @ ctn).argmax(axis=1)
    return bank_style[idx]


def kernel(content, bank_content, bank_style):
    # The axon PJRT plugin must be discoverable: a leftover JAX_PLATFORMS=cpu
    # (common when a harness pins the reference to CPU) would hide the
    # NeuronCores from jax. Only effective if jax isn't initialized yet.
    if os.environ.get("JAX_PLATFORMS") and \
            "axon" not in os.environ["JAX_PLATFORMS"]:
        import sys
        if "jax" not in sys.modules:
            del os.environ["JAX_PLATFORMS"]

    from concourse.bass_utils import run_bass_kernel_spmd

    content = np.ascontiguousarray(content, dtype=np.float32)
    bank_content = np.ascontiguousarray(bank_content, dtype=np.float32)
    bank_style = np.asarray(bank_style)
    cs = content.reshape(B, D)
    ct = bank_content.reshape(D, M)  # raw row-major reshape, NOT a transpose

    in_maps = []
    for c in range(NCORES):
        lo = c * DSH
        bank_pm = np.ascontiguousarray(
            ct[lo:lo + DSH].reshape(KT, 128, M).transpose(1, 0, 2).astype(FP8))
        in_maps.append({
            "qT": _make_qT(cs, lo),
            "bank": bank_pm,
        })

    nc = _get_nc()
    trace = bool(os.environ.get("BASSKNN_TRACE"))
    kwargs = {}
    if trace:
        _install_ntff_hook()
        kwargs = {"trace": True}
    res = None
    for attempt in range(3):
        try:
            res = run_bass_kernel_spmd(nc, in_maps, list(range(NCORES)),
                                       **kwargs)
            break
        except Exception:
            if attempt == 2:
                return _host_fallback(cs, ct, bank_style)
            kwargs = {}  # tracing is best-effort; never let it block results
            import time
            time.sleep(5)
    if trace:
        print(f"HW exec time: {res.exec_time_ns} ns")

    dot = np.zeros((B, M), np.float64)
    for c in range(NCORES):
        d = res.results[c]["dot_out"].astype(np.float64)
        for s in range(4):
            dot += d[0:64, s * M:(s + 1) * M] + d[64:128, s * M:(s + 1) * M]
    # exact f32 bank column norms, computed host-side (the device only needs
    # the fp8 dot; norms here cost one pass over bank_content in cache)
    ssq = np.einsum("dm,dm->m", ct, ct, dtype=np.float64)
    sim = dot / np.sqrt(ssq)[None, :]  # = cosine * ||cs_b||, per row b

    idx = sim.argmax(axis=1)
    # Exact re-rank of near-ties: any m whose fp8 sim is within
    # RERANK_MARGIN (cosine units) of the row max could be the true winner.
    row_norms = np.sqrt(np.einsum("bd,bd->b", cs, cs, dtype=np.float64))
    col_cache = {}
    for b in range(B):
        thr = RERANK_MARGIN * row_norms[b]
        cands = np.nonzero(sim[b] >= sim[b, idx[b]] - thr)[0]
        if len(cands) <= 1:
            continue
        row = cs[b].astype(np.float64)
        best_m, best_v = -1, -np.inf
        for m in sorted(int(x) for x in cands):
            if m not in col_cache:
                colf = ct[:, m].astype(np.float64)
                col_cache[m] = (colf, np.sqrt(colf @ colf))
            colf, nrm = col_cache[m]
            v = (row @ colf) / nrm
            if v > best_v:  # strict '>' keeps the lowest index on exact ties
                best_v, best_m = v, m
        idx[b] = best_m
    return bank_style[idx]



# revision 9
# speedup vs baseline: 1.1696x; 1.0331x over previous
"""Distributed 1-NN style-bank retrieval on 8 Trainium2 NeuronCores.

reference semantics:
    cs  = content.reshape(64, 524288), L2-normalized rows
    ct  = bank_content.reshape(524288, 256), L2-normalized cols
    idx = argmax(cs @ ct, axis=1);  out = bank_style[idx]

Strategy: shard the contraction axis D=524288 across the 8 cores (each core
reads every input byte exactly once — I/O optimal). Each core computes, in
fp8-e4m3 with f32 PSUM accumulation, partial dot[64, 256] = cs_shard @
ct_shard (query normalization cancels in the argmax, so it is skipped; bank
column norms are computed exactly on the host from the f32 data, so the
device streams nothing but the two fp8 operands). The host sums the 8 tiny
partials, forms sim = dot/sqrt(ssq), takes the argmax, and exactly re-ranks
(f64) any candidate within a safety margin of the winner — the margin is
~4.5x the measured fp8 perturbation, so the low-precision pass can never
silently flip a near-tie (the reference input contains a planted near-tie at
gap 1.2e-6, ~300x below the median gap).

Device schedule (measured on the 2-ring HWDGE: 16 DMA engines x ~25 GB/s
shared by the sync/SP and scalar/Activation trigger queues, 8 DMA
semaphores -> max 8 transfers in flight):
  - bank streams as 16 x 1 MiB blocks alternating rings (pieces much
    smaller than ~1 MiB shrink the in-flight window and bubble the
    stream; the last TWO blocks split into per-ring halves — measured
    the sweet spot: 0 or 3+ split blocks are several us worse);
  - query chunks (4 x 1 MiB) ride the opposite ring during the first
    four block slots;
  - the PE packs two k-tiles per PSUM column-group pair and accumulates
    into four bank-aligned PSUM segments drained progressively, so the
    final stop-matmul -> copy -> DMA chain covers only the last block;
  - dependency-free filler matmuls pad the PE during the early
    DMA-starved phase: the HAM activity monitor halves the PE clock in
    3413 ns epochs when the PE idles, and a half-clock window landing on
    the catch-up tail costs several us (fillers AFTER block 11 delay the
    real tail and measure ~5 us worse).
"""

import os

import numpy as np
import ml_dtypes

B, D, M, S = 64, 524288, 256, 2048
NCORES = 8
DSH = D // NCORES          # 65536 contraction rows per core
KT = DSH // 128            # 512 k-tiles of 128
G = int(os.environ.get("BASSKNN_G", "32"))   # k-tiles per bank DMA block
NBLK = KT // G
QCH = int(os.environ.get("BASSKNN_QCH", "128"))  # k-tiles per query chunk
NQCH = KT // QCH
RBUFS = int(os.environ.get("BASSKNN_RBUFS", "10"))
BF16 = ml_dtypes.bfloat16
FP8 = ml_dtypes.float8_e4m3

# |fp8 sim - exact sim| measured at 2.2e-4 (cosine units) on randn inputs of
# this shape; re-rank everything within ~4.5x that of the fp8 winner.
RERANK_MARGIN = 1e-3

_CACHED_NC = None


def _build_nc_v2():
    """Streaming-optimal schedule (v2).

    Trace analysis of v1 (71.45us) showed the 16-engine SDMA pool fully
    saturated (~424 GB/s) mid-stream, with all the loss at the edges:
      - first real matmul at 16.9us (query chunk 0 rode the scalar HWDGE
        ring, which starts ~3us after the sync ring and finished last);
      - three ~1us mid-stream stalls where dot_out drain DMAs sat at the
        head of a HWDGE FIFO, blocking later bank pieces behind a
        matmul-milestone wait (head-of-line blocking);
      - a 7us endgame ping-pong: the last bank half-blocks were queued
        behind drain DMAs whose matmuls needed exactly that bank data.
    v2 removes every mid-stream dependency from the two HWDGE rings:
      - the whole 16 MiB bank shard is SBUF-resident (one [128, KT, M]
        tile, pieces DMA'd into slices -> no buffer-reuse waits at all);
      - intermediate PSUM-segment drains go out the gpsimd SWDGE queue
        (third, independent FIFO) so the bank stream is never blocked;
        only the tiny final drain uses sync, after all bank DMAs;
      - tiny first pieces (query k-tiles 0:32 + bank 0:8) on the
        early-starting sync ring -> first real matmul at ~9.6us, and the
        PE (which then stays busy on backlog) HAM-warms by ~13us;
      - all query pieces ride sync: sync carries ~11 MB, scalar ~9 MB,
        compensating scalar's ~3us late ring start so both rings drain
        continuously and finish together;
      - final pieces are small so the last-block matmul tail after the
        final DMA byte is short.
    """
    import concourse.bacc as bacc
    import concourse.mybir as mybir
    from concourse import tile

    WU = int(os.environ.get("BASSKNN_WU", "8"))     # warmup fillers (HAM)
    F = int(os.environ.get("BASSKNN_F", "2"))       # fillers per bank piece
    NOFILL = int(os.environ.get("BASSKNN_NOFILL", "3"))  # no fillers on last N pieces

    # PSUM accumulation segments (k-tile boundaries, even, piece-aligned).
    SEG = [0, 192, 320, 448, KT]
    # Bank pieces (k-tile ranges): small head pieces so the PE starts
    # early, small tail pieces so the post-stream matmul tail is short.
    # EVERY piece is split between the two HWDGE rings (sync gets
    # `s` k-tiles, scalar the rest) so the per-k arrival pace of the two
    # rings is identical by construction; the head splits are sync-heavy
    # to absorb the scalar ring's measured ~3us later ring start. Mid
    # pieces are 64 kt so each ring's half keeps 8 KiB DMA descriptors
    # (the per-engine line-rate sweet spot).
    PIECES = []          # (lo, hi, sync_kt)
    sizes = [8, 16, 40] + [64] * 6 + [32, 16, 8, 8]
    splits = {0: 8, 1: 12, 2: 24}                  # sync share of head pieces
    lo = 0
    for i, sz in enumerate(sizes):
        s = splits.get(i, sz // 2)
        PIECES.append((lo, lo + sz, s))
        lo += sz
    assert lo == KT
    assert all((s % 2 == 0 or s == hi - lo) and 0 <= s <= hi - lo
               for lo, hi, s in PIECES)
    # Query chunks: chunk 0 is tiny, lands first, rides sync whole; later
    # chunks are split across both rings like the bank pieces, and each is
    # queued (in ring-FIFO order) just before the bank piece that first
    # needs it, so queries stay ahead of the matmul k-progress without
    # unbalancing the rings.
    QUERY = [(0, 32), (32, 160), (160, 288), (288, 416), (416, KT)]
    assert QUERY[0][0] == 0 and QUERY[-1][1] == KT
    assert all(a[1] == b[0] for a, b in zip(QUERY, QUERY[1:]))
    assert all(x in [p[0] for p in PIECES] + [KT] for x in SEG)

    nc = bacc.Bacc("TRN2", target_bir_lowering=False, debug=False,
                   num_devices=NCORES)
    qT = nc.dram_tensor("qT", [128, KT, B], mybir.dt.float8e4,
                        kind="ExternalInput")
    bank = nc.dram_tensor("bank", [128, KT, M], mybir.dt.float8e4,
                          kind="ExternalInput")
    dot_out = nc.dram_tensor("dot_out", [128, 4 * M], mybir.dt.float32,
                             kind="ExternalOutput")

    with tile.TileContext(nc) as tc:
        with tc.tile_pool(name="lhs", bufs=1) as plhs, \
             tc.tile_pool(name="rhs", bufs=1) as prhs, \
             tc.tile_pool(name="misc", bufs=1) as pmisc, \
             tc.tile_pool(name="psum", bufs=1, space="PSUM") as pps:
            ps_seg = [pps.tile([128, 512], mybir.dt.float32,
                               name=f"ps_seg{s}")
                      for s in range(4)]
            ps_warm = pps.tile([64, 512], mybir.dt.float32)
            lt = plhs.tile([128, KT, B], mybir.dt.float8e4)   # 32 KiB/part
            rt = prhs.tile([128, KT, M], mybir.dt.float8e4)   # 128 KiB/part
            dum = pmisc.tile([128, B], mybir.dt.float8e4)
            dumr = pmisc.tile([128, 512], mybir.dt.float8e4)
            dot_sb = pmisc.tile([128, 4 * M], mybir.dt.float32)

            # All streaming DMAs have no upstream dependencies -> both
            # HWDGE rings stream flat-out from the first instruction.
            # Build per-ring op lists: bank halves in k order, query
            # halves spliced in just before the bank piece at their k.
            def anchor(qlo):
                for i, (lo, hi, s) in enumerate(PIECES):
                    if lo <= qlo < hi:
                        return i
                return len(PIECES) - 1

            qat = {}             # piece index -> query chunk
            qat[0] = [QUERY[0]]
            for q in QUERY[1:]:
                qat.setdefault(anchor(q[0]), []).append(q)
            sync_ops, scal_ops = [], []
            for i, (lo, hi, s) in enumerate(PIECES):
                for (qlo, qhi) in qat.get(i, []):
                    if qlo == 0:             # tiny chunk 0: sync whole
                        sync_ops.append(("q", qlo, qhi))
                    else:
                        qm = (qlo + qhi) // 2
                        sync_ops.append(("q", qlo, qm))
                        scal_ops.append(("q", qm, qhi))
                if s > 0:
                    sync_ops.append(("b", lo, lo + s))
                if s < hi - lo:
                    scal_ops.append(("b", lo + s, hi))
            for ops, eng in ((sync_ops, nc.sync), (scal_ops, nc.scalar)):
                for kind, lo, hi in ops:
                    if kind == "q":
                        eng.dma_start(lt[:, lo:hi, :], qT[:, lo:hi, :])
                    else:
                        eng.dma_start(rt[:, lo:hi, :], bank[:, lo:hi, :])

            # Warmup burst: ~3.4us of dependency-free matmuls so the HAM
            # un-throttles the PE clock right as the first data lands.
            nc.any.memset(dum[:], 1.0)
            nc.any.memset(dumr[:], 1.0)
            for _ in range(WU):
                nc.tensor.matmul(ps_warm[:, :], dum[:], dumr[:],
                                 start=True, stop=True)

            def drain_seg(s, eng):
                nc.vector.tensor_copy(dot_sb[:, s * M:(s + 1) * M],
                                      ps_seg[s][:, :M])
                eng.dma_start(dot_out[:, s * M:(s + 1) * M],
                              dot_sb[:, s * M:(s + 1) * M])

            pieces = [(lo, hi) for lo, hi, s in PIECES]
            seg_idx = 0
            for pi, (plo, phi) in enumerate(pieces):
                for g in range(plo, phi):
                    half = 64 * (g % 2)
                    while g >= SEG[seg_idx + 1]:
                        seg_idx += 1
                    r0, r1 = SEG[seg_idx], SEG[seg_idx + 1]
                    nc.tensor.matmul(
                        ps_seg[seg_idx][half:half + 64, :M],
                        lt[:, g, :],
                        rt[:, g, :],
                        start=(g < r0 + 2),
                        stop=(g >= r1 - 2),
                    )
                    # Intermediate drains ride the gpsimd SWDGE queue so
                    # they can never block the HWDGE bank stream.
                    if g == SEG[1] - 1:
                        drain_seg(0, nc.gpsimd)
                    elif g == SEG[2] - 1:
                        drain_seg(1, nc.gpsimd)
                    elif g == SEG[3] - 1:
                        drain_seg(2, nc.gpsimd)
                if pi < len(pieces) - NOFILL:
                    for _ in range(F):
                        nc.tensor.matmul(ps_warm[:, :], dum[:], dumr[:],
                                         start=True, stop=True)
            # Final drain on sync: its queue is empty by now, and HWDGE
            # has the lower first-byte latency.
            drain_seg(3, nc.sync)
    nc.compile()
    return nc


def _build_nc_v1():
    import concourse.bacc as bacc
    import concourse.mybir as mybir
    from concourse import tile

    nc = bacc.Bacc("TRN2", target_bir_lowering=False, debug=False,
                   num_devices=NCORES)
    qT = nc.dram_tensor("qT", [128, KT, B], mybir.dt.float8e4,
                        kind="ExternalInput")
    bank = nc.dram_tensor("bank", [128, KT, M], mybir.dt.float8e4,
                          kind="ExternalInput")
    dot_out = nc.dram_tensor("dot_out", [128, 4 * M], mybir.dt.float32,
                             kind="ExternalOutput")

    with tile.TileContext(nc) as tc:
        with tc.tile_pool(name="lhs", bufs=1) as plhs, \
             tc.tile_pool(name="rhs", bufs=RBUFS) as prhs, \
             tc.tile_pool(name="misc", bufs=1) as pmisc, \
             tc.tile_pool(name="psum", bufs=1, space="PSUM") as pps:
            # four bank-aligned accumulators, drained progressively: the
            # copy+DMA of segments 0-2 overlap later compute, and the final
            # segment covers only the last bank block, so the end-of-kernel
            # serial chain (stop-matmul -> copy -> DMA) is as short as
            # possible. Segment s covers k-tiles [SEG[s], SEG[s+1]).
            SEG = [0, 192, 352, 480, KT]
            ps_seg = [pps.tile([128, 512], mybir.dt.float32,
                               name=f"ps_seg{s}")
                      for s in range(4)]
            ps_warm = pps.tile([64, 512], mybir.dt.float32)
            # all 512 query k-tiles stay resident (32 KiB/partition)
            lt = plhs.tile([128, KT, B], mybir.dt.float8e4)
            # Clock warm-up: the HAM starts the PE at half clock; a burst of
            # dependency-free matmuls during the (DMA-idle) ramp window keeps
            # it from throttling the first real blocks.
            dum = pmisc.tile([128, B], mybir.dt.float8e4)
            nc.any.memset(dum[:], 1.0)
            for _ in range(16):
                nc.tensor.matmul(ps_warm[:, 0:B], dum[:], dum[:],
                                 start=True, stop=True)
            # Only SP (sync) + Activation (scalar) can trigger HWDGE DMAs;
            # each queue drains at ~half the 16-engine pool rate and its
            # transfers complete in FIFO order. Schedule: ~1 MiB pieces
            # (small pieces shrink the 8-semaphore in-flight window and
            # bubble the stream), byte-balanced rings, query chunk blk on
            # the ring opposite bank block blk for blk < 4, and the last
            # two bank blocks split into per-ring halves so the stream's
            # final completion comes as early as possible on both rings.
            HALF = G // 2
            mm_done = 0
            seg_idx = 0

            def mm_block(blk, rt):
                nonlocal mm_done, seg_idx
                for j in range(G):
                    g = blk * G + j
                    # even k-tiles accumulate into PSUM partitions 0:64,
                    # odd into 64:128 (PE col-group packing — the two run
                    # concurrently); host adds the halves.
                    half = 64 * (g % 2)
                    while g >= SEG[seg_idx + 1]:
                        seg_idx += 1
                    r0, r1 = SEG[seg_idx], SEG[seg_idx + 1]
                    nc.tensor.matmul(
                        ps_seg[seg_idx][half:half + 64, :M],
                        lt[:, g, :],
                        rt[:, j, :],
                        start=(g < r0 + 2),
                        stop=(g >= r1 - 2),
                    )
                mm_done += G

            dot_sb = pmisc.tile([128, 4 * M], mybir.dt.float32)

            def drain_seg(s, eng):
                nc.vector.tensor_copy(dot_sb[:, s * M:(s + 1) * M],
                                      ps_seg[s][:, :M])
                eng.dma_start(dot_out[:, s * M:(s + 1) * M],
                              dot_sb[:, s * M:(s + 1) * M])

            def qt_piece(eng, t0, t1):
                eng.dma_start(lt[:, t0:t1, :], qT[:, t0:t1, :])

            def bank_half(eng, blk, rt, h):
                lo = blk * G + h * HALF
                eng.dma_start(rt[:, h * HALF:(h + 1) * HALF, :],
                              bank[:, lo:lo + HALF, :])

            def fillers(n):
                for _ in range(n):
                    nc.tensor.matmul(ps_warm[:, :], dum[:], lt[:, 0:8, :],
                                     start=True, stop=True)

            # Schedule knobs (A/B testing): v5 = champion filler layout
            # (3 per block through block 11, none after); v7 extends light
            # fillers through block 13 (measured ~5us worse); v9 tapers.
            SCHED = os.environ.get("BASSKNN_SCHED", "v5")
            for blk in range(NBLK):
                ring = nc.sync if blk % 2 == 0 else nc.scalar
                other = nc.scalar if blk % 2 == 0 else nc.sync
                if blk < NQCH:
                    qt_piece(other, blk * QCH, (blk + 1) * QCH)
                rt = prhs.tile([128, G, M], mybir.dt.float8e4, tag="rt")
                nsplit = {"v11": 3, "v12": 0}.get(SCHED, 2)
                if blk >= NBLK - nsplit:
                    bank_half(nc.sync, blk, rt, 0)
                    bank_half(nc.scalar, blk, rt, 1)
                else:
                    ring.dma_start(rt[:], bank[:, blk * G:(blk + 1) * G, :])
                mm_block(blk, rt)
                # HAM keep-warm: the activity monitor halves the PE clock
                # within ~2 epochs (3.4us each) of the PE going idle, and
                # the blocks arrive slower than the PE eats them; pad the
                # gaps with dependency-free matmuls, lighter near the end
                # so they never delay the final real work.
                if SCHED == "v9":
                    if blk < 6:
                        fillers(4)
                    elif blk < 10:
                        fillers(2)
                elif SCHED == "v10":
                    # oversize the early pads: the PE has ~8us of forced
                    # wait before block 3 lands, and idle there costs a
                    # half-clock HAM window later; surplus pad time is
                    # absorbed by the next block-arrival wait, so it is
                    # free everywhere except during the late catch-up.
                    if blk < 6:
                        fillers(8)
                    elif blk < 10:
                        fillers(4)
                    elif blk < 12:
                        fillers(2)
                elif blk < NBLK - 4:
                    fillers(3)
                elif SCHED == "v7" and blk < NBLK - 2:
                    fillers(2)
                if mm_done == SEG[1]:
                    drain_seg(0, nc.scalar)
                elif mm_done == SEG[2]:
                    drain_seg(1, nc.sync)
                elif mm_done == SEG[3]:
                    drain_seg(2, nc.scalar)
            drain_seg(3, nc.sync)
    nc.compile()
    return nc


def _get_nc():
    global _CACHED_NC
    if _CACHED_NC is None:
        impl = os.environ.get("BASSKNN_IMPL", "v2")
        _CACHED_NC = _build_nc_v1() if impl == "v1" else _build_nc_v2()
    return _CACHED_NC


def _make_qT(cs, lo):
    """[128, KT, B] fp8 with qT[p, t, b] = cs[b, lo + t*128 + p]."""
    csT = np.empty((DSH, B), FP8)
    BLK = 4096  # 64 x 4096 x 4B = 1 MiB working set per block
    sub = cs[:, lo:lo + DSH]
    for j in range(0, DSH, BLK):
        csT[j:j + BLK] = sub[:, j:j + BLK].T
    return np.ascontiguousarray(csT.reshape(KT, 128, B).transpose(1, 0, 2))


def _install_ntff_hook():
    """Register the axon NTFF profile hook missing from this image's antenv
    (profiling path only — used when BASSKNN_TRACE=1)."""
    import contextlib
    import ctypes
    import sys
    import types

    if "antenv.axon_hooks" in sys.modules:
        return
    lib = ctypes.CDLL("/opt/axon/libaxon_pjrt.so")
    lib.axon_start_nrt_profile.argtypes = [ctypes.POINTER(ctypes.c_int64),
                                           ctypes.c_size_t]
    lib.axon_start_nrt_profile.restype = ctypes.c_int64
    lib.axon_stop_nrt_profile.argtypes = [ctypes.c_char_p]
    lib.axon_stop_nrt_profile.restype = ctypes.c_int64

    @contextlib.contextmanager
    def _hook(output_dir, device_ids):
        import jax

        jax.devices()
        if device_ids:
            ids = (ctypes.c_int64 * len(device_ids))(*device_ids)
            rc = lib.axon_start_nrt_profile(ids, len(device_ids))
        else:
            rc = lib.axon_start_nrt_profile(None, 0)
        if rc != 0:
            raise RuntimeError(f"axon_start_nrt_profile rc={rc}")
        try:
            yield
        finally:
            n = lib.axon_stop_nrt_profile(str(output_dir).encode())
            print(f"ntff profile: {n} file(s) -> {output_dir}", file=sys.stderr)

    mod = types.ModuleType("antenv.axon_hooks")
    mod.get_axon_ntff_profile_hook = lambda: _hook
    sys.modules["antenv.axon_hooks"] = mod
    import concourse.bass_utils as bass_utils

    bass_utils.upload_artifacts = lambda tmpdir: "local://" + tmpdir


def _host_fallback(cs, ct, bank_style):
    """Pure-numpy emergency path (device unavailable): exact reference math."""
    cs64 = cs.astype(np.float64)
    ct64 = ct.astype(np.float64)
    csn = cs64 / np.maximum(np.linalg.norm(cs64, axis=1, keepdims=True), 1e-12)
    ctn = ct64 / np.maximum(np.linalg.norm(ct64, axis=0, keepdims=True), 1e-12)
    idx = (csn @ ctn).argmax(axis=1)
    return bank_style[idx]


def kernel(content, bank_content, bank_style):
    # The axon PJRT plugin must be discoverable: a leftover JAX_PLATFORMS=cpu
    # (common when a harness pins the reference to CPU) would hide the
    # NeuronCores from jax. Only effective if jax isn't initialized yet.
    if os.environ.get("JAX_PLATFORMS") and \
            "axon" not in os.environ["JAX_PLATFORMS"]:
        import sys
        if "jax" not in sys.modules:
            del os.environ["JAX_PLATFORMS"]

    from concourse.bass_utils import run_bass_kernel_spmd

    content = np.ascontiguousarray(content, dtype=np.float32)
    bank_content = np.ascontiguousarray(bank_content, dtype=np.float32)
    bank_style = np.asarray(bank_style)
    cs = content.reshape(B, D)
    ct = bank_content.reshape(D, M)  # raw row-major reshape, NOT a transpose

    in_maps = []
    for c in range(NCORES):
        lo = c * DSH
        bank_pm = np.ascontiguousarray(
            ct[lo:lo + DSH].reshape(KT, 128, M).transpose(1, 0, 2).astype(FP8))
        in_maps.append({
            "qT": _make_qT(cs, lo),
            "bank": bank_pm,
        })

    nc = _get_nc()
    trace = bool(os.environ.get("BASSKNN_TRACE"))
    kwargs = {}
    if trace:
        _install_ntff_hook()
        kwargs = {"trace": True}
    res = None
    for attempt in range(3):
        try:
            res = run_bass_kernel_spmd(nc, in_maps, list(range(NCORES)),
                                       **kwargs)
            break
        except Exception:
            if attempt == 2:
                return _host_fallback(cs, ct, bank_style)
            kwargs = {}  # tracing is best-effort; never let it block results
            import time
            time.sleep(5)
    if trace:
        print(f"HW exec time: {res.exec_time_ns} ns")

    dot = np.zeros((B, M), np.float64)
    for c in range(NCORES):
        d = res.results[c]["dot_out"].astype(np.float64)
        for s in range(4):
            dot += d[0:64, s * M:(s + 1) * M] + d[64:128, s * M:(s + 1) * M]
    # exact f32 bank column norms, computed host-side (the device only needs
    # the fp8 dot; norms here cost one pass over bank_content in cache)
    ssq = np.einsum("dm,dm->m", ct, ct, dtype=np.float64)
    sim = dot / np.sqrt(ssq)[None, :]  # = cosine * ||cs_b||, per row b

    idx = sim.argmax(axis=1)
    # Exact re-rank of near-ties: any m whose fp8 sim is within
    # RERANK_MARGIN (cosine units) of the row max could be the true winner.
    row_norms = np.sqrt(np.einsum("bd,bd->b", cs, cs, dtype=np.float64))
    col_cache = {}
    for b in range(B):
        thr = RERANK_MARGIN * row_norms[b]
        cands = np.nonzero(sim[b] >= sim[b, idx[b]] - thr)[0]
        if len(cands) <= 1:
            continue
        row = cs[b].astype(np.float64)
        best_m, best_v = -1, -np.inf
        for m in sorted(int(x) for x in cands):
            if m not in col_cache:
                colf = ct[:, m].astype(np.float64)
                col_cache[m] = (colf, np.sqrt(colf @ colf))
            colf, nrm = col_cache[m]
            v = (row @ colf) / nrm
            if v > best_v:  # strict '>' keeps the lowest index on exact ties
                best_v, best_m = v, m
        idx[b] = best_m
    return bank_style[idx]



# revision 10
# speedup vs baseline: 1.2771x; 1.0920x over previous
"""Distributed 1-NN style-bank retrieval on 8 Trainium2 NeuronCores.

reference semantics:
    cs  = content.reshape(64, 524288), L2-normalized rows
    ct  = bank_content.reshape(524288, 256), L2-normalized cols
    idx = argmax(cs @ ct, axis=1);  out = bank_style[idx]

Strategy: shard the contraction axis D=524288 across the 8 cores (each core
reads every input byte exactly once — I/O optimal). Each core computes, in
fp8-e4m3 with f32 PSUM accumulation, partial dot[64, 256] = cs_shard @
ct_shard (query normalization cancels in the argmax, so it is skipped; bank
column norms are computed exactly on the host from the f32 data, so the
device streams nothing but the two fp8 operands). The host sums the 8 tiny
partials, forms sim = dot/sqrt(ssq), takes the argmax, and exactly re-ranks
(f64) any candidate within a safety margin of the winner — the margin is
~4.5x the measured fp8 perturbation, so the low-precision pass can never
silently flip a near-tie (the reference input contains a planted near-tie at
gap 1.2e-6, ~300x below the median gap).

Device schedule (measured on the 2-ring HWDGE: 16 DMA engines x ~25 GB/s
shared by the sync/SP and scalar/Activation trigger queues, 8 DMA
semaphores -> max 8 transfers in flight):
  - bank streams as 16 x 1 MiB blocks alternating rings (pieces much
    smaller than ~1 MiB shrink the in-flight window and bubble the
    stream; the last TWO blocks split into per-ring halves — measured
    the sweet spot: 0 or 3+ split blocks are several us worse);
  - query chunks (4 x 1 MiB) ride the opposite ring during the first
    four block slots;
  - the PE packs two k-tiles per PSUM column-group pair and accumulates
    into four bank-aligned PSUM segments drained progressively, so the
    final stop-matmul -> copy -> DMA chain covers only the last block;
  - dependency-free filler matmuls pad the PE during the early
    DMA-starved phase: the HAM activity monitor halves the PE clock in
    3413 ns epochs when the PE idles, and a half-clock window landing on
    the catch-up tail costs several us (fillers AFTER block 11 delay the
    real tail and measure ~5 us worse).
"""

import os

import numpy as np
import ml_dtypes

B, D, M, S = 64, 524288, 256, 2048
NCORES = 8
DSH = D // NCORES          # 65536 contraction rows per core
KT = DSH // 128            # 512 k-tiles of 128
G = int(os.environ.get("BASSKNN_G", "32"))   # k-tiles per bank DMA block
NBLK = KT // G
QCH = int(os.environ.get("BASSKNN_QCH", "128"))  # k-tiles per query chunk
NQCH = KT // QCH
RBUFS = int(os.environ.get("BASSKNN_RBUFS", "10"))
BF16 = ml_dtypes.bfloat16
FP8 = ml_dtypes.float8_e4m3

# |fp8 sim - exact sim| measured at 2.2e-4 (cosine units) on randn inputs of
# this shape; re-rank everything within ~4.5x that of the fp8 winner.
RERANK_MARGIN = 1e-3

_CACHED_NC = None


def _build_nc_v2():
    """Single-ring streaming schedule (v2.3).

    Evidence from the v1 trace (71.45us):
      - one HWDGE ring alone saturates all 16 SDMA engines (~424 GB/s):
        queue 1 had 16/16 engines busy before queue 10's ring woke up;
      - the scalar ring starts ~3us after sync, so anything critical on
        it (v1: query chunk 0 -> first matmul at 16.9us) arrives late;
      - dot_out drain DMAs in a streaming ring head-of-line block later
        bank pieces behind matmul-milestone waits (v1 lost ~7us at the
        end to a drain <-> matmul ping-pong);
      - the Tile scheduler's 8 DMAHW sem lanes make DMA issue wait on
        the completion of the DMA 8 places earlier, so the streaming
        instruction count must stay ~25ish and pieces ~1 MiB (8 KiB
        per-partition descriptors) or the issue pace caps the stream.
    Design: ALL input bytes flow through the sync ring as one FIFO in
    exact k-interleaved order (queries spliced ahead of the bank pieces
    that need them); the whole bank shard is SBUF-resident (no reuse
    deps); the four dot_out drains ride the otherwise-empty scalar ring;
    small head pieces start the PE at ~11us and small tail pieces keep
    the post-stream matmul tail short; warmup + filler matmuls hold the
    PE clock at full rate (HAM) through the DMA-paced middle.
    """
    import concourse.bacc as bacc
    import concourse.mybir as mybir
    from concourse import tile

    WU = int(os.environ.get("BASSKNN_WU", "8"))     # warmup fillers (HAM)
    F = int(os.environ.get("BASSKNN_F", "2"))       # fillers per bank piece
    NOFILL = int(os.environ.get("BASSKNN_NOFILL", "3"))

    # PSUM accumulation segments (k-tile boundaries, even).
    SEG = [0, 192, 352, 480, KT]
    # Single-ring stream: (kind, lo, hi) in exact FIFO order. Queries ride
    # one step ahead of the bank k-range that needs them.
    STREAM = [
        ("q", 0, 16), ("b", 0, 8), ("q", 16, 144), ("b", 8, 32),
        ("b", 32, 64), ("q", 144, 272), ("b", 64, 96), ("b", 96, 128),
        ("q", 272, 400), ("b", 128, 160), ("b", 160, 192),
        ("q", 400, 512), ("b", 192, 224), ("b", 224, 256),
        ("b", 256, 288), ("b", 288, 320), ("b", 320, 352),
        ("b", 352, 384), ("b", 384, 416), ("b", 416, 448),
        ("b", 448, 480), ("b", 480, 496), ("b", 496, 512),
    ]
    bank_pieces = [(lo, hi) for k, lo, hi in STREAM if k == "b"]
    qcov = [(lo, hi) for k, lo, hi in STREAM if k == "q"]
    assert bank_pieces[0][0] == 0 and bank_pieces[-1][1] == KT
    assert all(a[1] == b[0] for a, b in zip(bank_pieces, bank_pieces[1:]))
    assert qcov[0][0] == 0 and qcov[-1][1] == KT
    assert all(a[1] == b[0] for a, b in zip(qcov, qcov[1:]))
    # every query chunk must be issued before the bank piece containing
    # its first k-tile
    for qi, (k, lo, hi) in enumerate(STREAM):
        if k == "q" and lo > 0:
            later_bank = [l for kk, l, h in STREAM[qi + 1:] if kk == "b"]
            assert later_bank and min(later_bank) <= lo, (lo, hi)

    nc = bacc.Bacc("TRN2", target_bir_lowering=False, debug=False,
                   num_devices=NCORES)
    qT = nc.dram_tensor("qT", [128, KT, B], mybir.dt.float8e4,
                        kind="ExternalInput")
    bank = nc.dram_tensor("bank", [128, KT, M], mybir.dt.float8e4,
                          kind="ExternalInput")
    dot_out = nc.dram_tensor("dot_out", [128, 4 * M], mybir.dt.float32,
                             kind="ExternalOutput")

    with tile.TileContext(nc) as tc:
        with tc.tile_pool(name="lhs", bufs=1) as plhs, \
             tc.tile_pool(name="rhs", bufs=1) as prhs, \
             tc.tile_pool(name="misc", bufs=1) as pmisc, \
             tc.tile_pool(name="psum", bufs=1, space="PSUM") as pps:
            ps_seg = [pps.tile([128, 512], mybir.dt.float32,
                               name=f"ps_seg{s}")
                      for s in range(4)]
            ps_warm = pps.tile([64, 512], mybir.dt.float32)
            lt = plhs.tile([128, KT, B], mybir.dt.float8e4)   # 32 KiB/part
            rt = prhs.tile([128, KT, M], mybir.dt.float8e4)   # 128 KiB/part
            dum = pmisc.tile([128, B], mybir.dt.float8e4)
            dumr = pmisc.tile([128, 512], mybir.dt.float8e4)
            dot_sb = pmisc.tile([128, 4 * M], mybir.dt.float32)

            # The entire input stream, one FIFO, no dependencies.
            for kind, lo, hi in STREAM:
                if kind == "q":
                    nc.sync.dma_start(lt[:, lo:hi, :], qT[:, lo:hi, :])
                else:
                    nc.sync.dma_start(rt[:, lo:hi, :], bank[:, lo:hi, :])

            # Warmup burst: ~3.4us of dependency-free matmuls so the HAM
            # un-throttles the PE clock right as the first data lands.
            nc.any.memset(dum[:], 1.0)
            nc.any.memset(dumr[:], 1.0)
            for _ in range(WU):
                nc.tensor.matmul(ps_warm[:, :], dum[:], dumr[:],
                                 start=True, stop=True)

            def drain_seg(s):
                # scalar ring carries only these tiny drains -> they can
                # never block the input stream.
                nc.vector.tensor_copy(dot_sb[:, s * M:(s + 1) * M],
                                      ps_seg[s][:, :M])
                nc.scalar.dma_start(dot_out[:, s * M:(s + 1) * M],
                                    dot_sb[:, s * M:(s + 1) * M])

            seg_idx = 0
            for pi, (plo, phi) in enumerate(bank_pieces):
                for g in range(plo, phi):
                    half = 64 * (g % 2)
                    while g >= SEG[seg_idx + 1]:
                        seg_idx += 1
                    r0, r1 = SEG[seg_idx], SEG[seg_idx + 1]
                    nc.tensor.matmul(
                        ps_seg[seg_idx][half:half + 64, :M],
                        lt[:, g, :],
                        rt[:, g, :],
                        start=(g < r0 + 2),
                        stop=(g >= r1 - 2),
                    )
                    if g in (SEG[1] - 1, SEG[2] - 1, SEG[3] - 1):
                        drain_seg(SEG.index(g + 1) - 1)
                if pi < len(bank_pieces) - NOFILL:
                    for _ in range(F):
                        nc.tensor.matmul(ps_warm[:, :], dum[:], dumr[:],
                                         start=True, stop=True)
            drain_seg(3)
    nc.compile()
    return nc


def _build_nc_v1():
    import concourse.bacc as bacc
    import concourse.mybir as mybir
    from concourse import tile

    nc = bacc.Bacc("TRN2", target_bir_lowering=False, debug=False,
                   num_devices=NCORES)
    qT = nc.dram_tensor("qT", [128, KT, B], mybir.dt.float8e4,
                        kind="ExternalInput")
    bank = nc.dram_tensor("bank", [128, KT, M], mybir.dt.float8e4,
                          kind="ExternalInput")
    dot_out = nc.dram_tensor("dot_out", [128, 4 * M], mybir.dt.float32,
                             kind="ExternalOutput")

    with tile.TileContext(nc) as tc:
        with tc.tile_pool(name="lhs", bufs=1) as plhs, \
             tc.tile_pool(name="rhs", bufs=RBUFS) as prhs, \
             tc.tile_pool(name="misc", bufs=1) as pmisc, \
             tc.tile_pool(name="psum", bufs=1, space="PSUM") as pps:
            # four bank-aligned accumulators, drained progressively: the
            # copy+DMA of segments 0-2 overlap later compute, and the final
            # segment covers only the last bank block, so the end-of-kernel
            # serial chain (stop-matmul -> copy -> DMA) is as short as
            # possible. Segment s covers k-tiles [SEG[s], SEG[s+1]).
            SEG = [0, 192, 352, 480, KT]
            ps_seg = [pps.tile([128, 512], mybir.dt.float32,
                               name=f"ps_seg{s}")
                      for s in range(4)]
            ps_warm = pps.tile([64, 512], mybir.dt.float32)
            # all 512 query k-tiles stay resident (32 KiB/partition)
            lt = plhs.tile([128, KT, B], mybir.dt.float8e4)
            # Clock warm-up: the HAM starts the PE at half clock; a burst of
            # dependency-free matmuls during the (DMA-idle) ramp window keeps
            # it from throttling the first real blocks.
            dum = pmisc.tile([128, B], mybir.dt.float8e4)
            nc.any.memset(dum[:], 1.0)
            for _ in range(16):
                nc.tensor.matmul(ps_warm[:, 0:B], dum[:], dum[:],
                                 start=True, stop=True)
            # Only SP (sync) + Activation (scalar) can trigger HWDGE DMAs;
            # each queue drains at ~half the 16-engine pool rate and its
            # transfers complete in FIFO order. Schedule: ~1 MiB pieces
            # (small pieces shrink the 8-semaphore in-flight window and
            # bubble the stream), byte-balanced rings, query chunk blk on
            # the ring opposite bank block blk for blk < 4, and the last
            # two bank blocks split into per-ring halves so the stream's
            # final completion comes as early as possible on both rings.
            HALF = G // 2
            mm_done = 0
            seg_idx = 0

            def mm_block(blk, rt):
                nonlocal mm_done, seg_idx
                for j in range(G):
                    g = blk * G + j
                    # even k-tiles accumulate into PSUM partitions 0:64,
                    # odd into 64:128 (PE col-group packing — the two run
                    # concurrently); host adds the halves.
                    half = 64 * (g % 2)
                    while g >= SEG[seg_idx + 1]:
                        seg_idx += 1
                    r0, r1 = SEG[seg_idx], SEG[seg_idx + 1]
                    nc.tensor.matmul(
                        ps_seg[seg_idx][half:half + 64, :M],
                        lt[:, g, :],
                        rt[:, j, :],
                        start=(g < r0 + 2),
                        stop=(g >= r1 - 2),
                    )
                mm_done += G

            dot_sb = pmisc.tile([128, 4 * M], mybir.dt.float32)

            def drain_seg(s, eng):
                nc.vector.tensor_copy(dot_sb[:, s * M:(s + 1) * M],
                                      ps_seg[s][:, :M])
                eng.dma_start(dot_out[:, s * M:(s + 1) * M],
                              dot_sb[:, s * M:(s + 1) * M])

            def qt_piece(eng, t0, t1):
                eng.dma_start(lt[:, t0:t1, :], qT[:, t0:t1, :])

            def bank_half(eng, blk, rt, h):
                lo = blk * G + h * HALF
                eng.dma_start(rt[:, h * HALF:(h + 1) * HALF, :],
                              bank[:, lo:lo + HALF, :])

            def fillers(n):
                for _ in range(n):
                    nc.tensor.matmul(ps_warm[:, :], dum[:], lt[:, 0:8, :],
                                     start=True, stop=True)

            # Schedule knobs (A/B testing): v5 = champion filler layout
            # (3 per block through block 11, none after); v7 extends light
            # fillers through block 13 (measured ~5us worse); v9 tapers.
            SCHED = os.environ.get("BASSKNN_SCHED", "v5")
            for blk in range(NBLK):
                ring = nc.sync if blk % 2 == 0 else nc.scalar
                other = nc.scalar if blk % 2 == 0 else nc.sync
                if blk < NQCH:
                    qt_piece(other, blk * QCH, (blk + 1) * QCH)
                rt = prhs.tile([128, G, M], mybir.dt.float8e4, tag="rt")
                nsplit = {"v11": 3, "v12": 0}.get(SCHED, 2)
                if blk >= NBLK - nsplit:
                    bank_half(nc.sync, blk, rt, 0)
                    bank_half(nc.scalar, blk, rt, 1)
                else:
                    ring.dma_start(rt[:], bank[:, blk * G:(blk + 1) * G, :])
                mm_block(blk, rt)
                # HAM keep-warm: the activity monitor halves the PE clock
                # within ~2 epochs (3.4us each) of the PE going idle, and
                # the blocks arrive slower than the PE eats them; pad the
                # gaps with dependency-free matmuls, lighter near the end
                # so they never delay the final real work.
                if SCHED == "v9":
                    if blk < 6:
                        fillers(4)
                    elif blk < 10:
                        fillers(2)
                elif SCHED == "v10":
                    # oversize the early pads: the PE has ~8us of forced
                    # wait before block 3 lands, and idle there costs a
                    # half-clock HAM window later; surplus pad time is
                    # absorbed by the next block-arrival wait, so it is
                    # free everywhere except during the late catch-up.
                    if blk < 6:
                        fillers(8)
                    elif blk < 10:
                        fillers(4)
                    elif blk < 12:
                        fillers(2)
                elif blk < NBLK - 4:
                    fillers(3)
                elif SCHED == "v7" and blk < NBLK - 2:
                    fillers(2)
                if mm_done == SEG[1]:
                    drain_seg(0, nc.scalar)
                elif mm_done == SEG[2]:
                    drain_seg(1, nc.sync)
                elif mm_done == SEG[3]:
                    drain_seg(2, nc.scalar)
            drain_seg(3, nc.sync)
    nc.compile()
    return nc


def _get_nc():
    global _CACHED_NC
    if _CACHED_NC is None:
        impl = os.environ.get("BASSKNN_IMPL", "v2")
        _CACHED_NC = _build_nc_v1() if impl == "v1" else _build_nc_v2()
    return _CACHED_NC


def _make_qT(cs, lo):
    """[128, KT, B] fp8 with qT[p, t, b] = cs[b, lo + t*128 + p]."""
    csT = np.empty((DSH, B), FP8)
    BLK = 4096  # 64 x 4096 x 4B = 1 MiB working set per block
    sub = cs[:, lo:lo + DSH]
    for j in range(0, DSH, BLK):
        csT[j:j + BLK] = sub[:, j:j + BLK].T
    return np.ascontiguousarray(csT.reshape(KT, 128, B).transpose(1, 0, 2))


def _install_ntff_hook():
    """Register the axon NTFF profile hook missing from this image's antenv
    (profiling path only — used when BASSKNN_TRACE=1)."""
    import contextlib
    import ctypes
    import sys
    import types

    if "antenv.axon_hooks" in sys.modules:
        return
    lib = ctypes.CDLL("/opt/axon/libaxon_pjrt.so")
    lib.axon_start_nrt_profile.argtypes = [ctypes.POINTER(ctypes.c_int64),
                                           ctypes.c_size_t]
    lib.axon_start_nrt_profile.restype = ctypes.c_int64
    lib.axon_stop_nrt_profile.argtypes = [ctypes.c_char_p]
    lib.axon_stop_nrt_profile.restype = ctypes.c_int64

    @contextlib.contextmanager
    def _hook(output_dir, device_ids):
        import jax

        jax.devices()
        if device_ids:
            ids = (ctypes.c_int64 * len(device_ids))(*device_ids)
            rc = lib.axon_start_nrt_profile(ids, len(device_ids))
        else:
            rc = lib.axon_start_nrt_profile(None, 0)
        if rc != 0:
            raise RuntimeError(f"axon_start_nrt_profile rc={rc}")
        try:
            yield
        finally:
            n = lib.axon_stop_nrt_profile(str(output_dir).encode())
            print(f"ntff profile: {n} file(s) -> {output_dir}", file=sys.stderr)

    mod = types.ModuleType("antenv.axon_hooks")
    mod.get_axon_ntff_profile_hook = lambda: _hook
    sys.modules["antenv.axon_hooks"] = mod
    import concourse.bass_utils as bass_utils

    bass_utils.upload_artifacts = lambda tmpdir: "local://" + tmpdir


def _host_fallback(cs, ct, bank_style):
    """Pure-numpy emergency path (device unavailable): exact reference math."""
    cs64 = cs.astype(np.float64)
    ct64 = ct.astype(np.float64)
    csn = cs64 / np.maximum(np.linalg.norm(cs64, axis=1, keepdims=True), 1e-12)
    ctn = ct64 / np.maximum(np.linalg.norm(ct64, axis=0, keepdims=True), 1e-12)
    idx = (csn @ ctn).argmax(axis=1)
    return bank_style[idx]


def kernel(content, bank_content, bank_style):
    # The axon PJRT plugin must be discoverable: a leftover JAX_PLATFORMS=cpu
    # (common when a harness pins the reference to CPU) would hide the
    # NeuronCores from jax. Only effective if jax isn't initialized yet.
    if os.environ.get("JAX_PLATFORMS") and \
            "axon" not in os.environ["JAX_PLATFORMS"]:
        import sys
        if "jax" not in sys.modules:
            del os.environ["JAX_PLATFORMS"]

    from concourse.bass_utils import run_bass_kernel_spmd

    content = np.ascontiguousarray(content, dtype=np.float32)
    bank_content = np.ascontiguousarray(bank_content, dtype=np.float32)
    bank_style = np.asarray(bank_style)
    cs = content.reshape(B, D)
    ct = bank_content.reshape(D, M)  # raw row-major reshape, NOT a transpose

    in_maps = []
    for c in range(NCORES):
        lo = c * DSH
        bank_pm = np.ascontiguousarray(
            ct[lo:lo + DSH].reshape(KT, 128, M).transpose(1, 0, 2).astype(FP8))
        in_maps.append({
            "qT": _make_qT(cs, lo),
            "bank": bank_pm,
        })

    nc = _get_nc()
    trace = bool(os.environ.get("BASSKNN_TRACE"))
    kwargs = {}
    if trace:
        _install_ntff_hook()
        kwargs = {"trace": True}
    res = None
    for attempt in range(3):
        try:
            res = run_bass_kernel_spmd(nc, in_maps, list(range(NCORES)),
                                       **kwargs)
            break
        except Exception:
            if attempt == 2:
                return _host_fallback(cs, ct, bank_style)
            kwargs = {}  # tracing is best-effort; never let it block results
            import time
            time.sleep(5)
    if trace:
        print(f"HW exec time: {res.exec_time_ns} ns")

    dot = np.zeros((B, M), np.float64)
    for c in range(NCORES):
        d = res.results[c]["dot_out"].astype(np.float64)
        for s in range(4):
            dot += d[0:64, s * M:(s + 1) * M] + d[64:128, s * M:(s + 1) * M]
    # exact f32 bank column norms, computed host-side (the device only needs
    # the fp8 dot; norms here cost one pass over bank_content in cache)
    ssq = np.einsum("dm,dm->m", ct, ct, dtype=np.float64)
    sim = dot / np.sqrt(ssq)[None, :]  # = cosine * ||cs_b||, per row b

    idx = sim.argmax(axis=1)
    # Exact re-rank of near-ties: any m whose fp8 sim is within
    # RERANK_MARGIN (cosine units) of the row max could be the true winner.
    row_norms = np.sqrt(np.einsum("bd,bd->b", cs, cs, dtype=np.float64))
    col_cache = {}
    for b in range(B):
        thr = RERANK_MARGIN * row_norms[b]
        cands = np.nonzero(sim[b] >= sim[b, idx[b]] - thr)[0]
        if len(cands) <= 1:
            continue
        row = cs[b].astype(np.float64)
        best_m, best_v = -1, -np.inf
        for m in sorted(int(x) for x in cands):
            if m not in col_cache:
                colf = ct[:, m].astype(np.float64)
                col_cache[m] = (colf, np.sqrt(colf @ colf))
            colf, nrm = col_cache[m]
            v = (row @ colf) / nrm
            if v > best_v:  # strict '>' keeps the lowest index on exact ties
                best_v, best_m = v, m
        idx[b] = best_m
    return bank_style[idx]



# revision 13
# speedup vs baseline: 1.2833x; 1.0048x over previous
"""Distributed 1-NN style-bank retrieval on 8 Trainium2 NeuronCores.

reference semantics:
    cs  = content.reshape(64, 524288), L2-normalized rows
    ct  = bank_content.reshape(524288, 256), L2-normalized cols
    idx = argmax(cs @ ct, axis=1);  out = bank_style[idx]

Strategy: shard the contraction axis D=524288 across the 8 cores (each core
reads every input byte exactly once — I/O optimal). Each core computes, in
fp8-e4m3 with f32 PSUM accumulation, partial dot[64, 256] = cs_shard @
ct_shard (query normalization cancels in the argmax, so it is skipped; bank
column norms are computed exactly on the host from the f32 data, so the
device streams nothing but the two fp8 operands). The host sums the 8 tiny
partials, forms sim = dot/sqrt(ssq), takes the argmax, and exactly re-ranks
(f64) any candidate within a safety margin of the winner — the margin is
~4.5x the measured fp8 perturbation, so the low-precision pass can never
silently flip a near-tie (the reference input contains a planted near-tie at
gap 1.2e-6, ~300x below the median gap).

Device schedule (measured on the 2-ring HWDGE: 16 DMA engines x ~25 GB/s
shared by the sync/SP and scalar/Activation trigger queues, 8 DMA
semaphores -> max 8 transfers in flight):
  - bank streams as 16 x 1 MiB blocks alternating rings (pieces much
    smaller than ~1 MiB shrink the in-flight window and bubble the
    stream; the last TWO blocks split into per-ring halves — measured
    the sweet spot: 0 or 3+ split blocks are several us worse);
  - query chunks (4 x 1 MiB) ride the opposite ring during the first
    four block slots;
  - the PE packs two k-tiles per PSUM column-group pair and accumulates
    into four bank-aligned PSUM segments drained progressively, so the
    final stop-matmul -> copy -> DMA chain covers only the last block;
  - dependency-free filler matmuls pad the PE during the early
    DMA-starved phase: the HAM activity monitor halves the PE clock in
    3413 ns epochs when the PE idles, and a half-clock window landing on
    the catch-up tail costs several us (fillers AFTER block 11 delay the
    real tail and measure ~5 us worse).
"""

import os

import numpy as np
import ml_dtypes

B, D, M, S = 64, 524288, 256, 2048
NCORES = 8
DSH = D // NCORES          # 65536 contraction rows per core
KT = DSH // 128            # 512 k-tiles of 128
G = int(os.environ.get("BASSKNN_G", "32"))   # k-tiles per bank DMA block
NBLK = KT // G
QCH = int(os.environ.get("BASSKNN_QCH", "128"))  # k-tiles per query chunk
NQCH = KT // QCH
RBUFS = int(os.environ.get("BASSKNN_RBUFS", "10"))
BF16 = ml_dtypes.bfloat16
FP8 = ml_dtypes.float8_e4m3

# |fp8 sim - exact sim| measured at 2.2e-4 (cosine units) on randn inputs of
# this shape; re-rank everything within ~4.5x that of the fp8 winner.
RERANK_MARGIN = 1e-3

_CACHED_NC = None


def _build_nc_v2():
    """Single-ring streaming schedule (v2.3).

    Evidence from the v1 trace (71.45us):
      - one HWDGE ring alone saturates all 16 SDMA engines (~424 GB/s):
        queue 1 had 16/16 engines busy before queue 10's ring woke up;
      - the scalar ring starts ~3us after sync, so anything critical on
        it (v1: query chunk 0 -> first matmul at 16.9us) arrives late;
      - dot_out drain DMAs in a streaming ring head-of-line block later
        bank pieces behind matmul-milestone waits (v1 lost ~7us at the
        end to a drain <-> matmul ping-pong);
      - the Tile scheduler's 8 DMAHW sem lanes make DMA issue wait on
        the completion of the DMA 8 places earlier, so the streaming
        instruction count must stay ~25ish and pieces ~1 MiB (8 KiB
        per-partition descriptors) or the issue pace caps the stream.
    Design: ALL input bytes flow through the sync ring as one FIFO in
    exact k-interleaved order (queries spliced ahead of the bank pieces
    that need them); the whole bank shard is SBUF-resident (no reuse
    deps); the four dot_out drains ride the otherwise-empty scalar ring;
    small head pieces start the PE at ~11us and small tail pieces keep
    the post-stream matmul tail short; warmup + filler matmuls hold the
    PE clock at full rate (HAM) through the DMA-paced middle.
    """
    import concourse.bacc as bacc
    import concourse.mybir as mybir
    from concourse import tile

    WU = int(os.environ.get("BASSKNN_WU", "8"))     # warmup fillers (HAM)
    F = int(os.environ.get("BASSKNN_F", "2"))       # fillers per bank piece
    NOFILL = int(os.environ.get("BASSKNN_NOFILL", "3"))

    # PSUM accumulation segments (k-tile boundaries, even).
    SEG = [0, 192, 352, 480, KT]
    # Single-ring stream: (kind, lo, hi) in exact FIFO order. Queries ride
    # one step ahead of the bank k-range that needs them.
    STREAM = [
        ("q", 0, 16), ("b", 0, 8), ("q", 16, 144), ("b", 8, 32),
        ("b", 32, 64), ("q", 144, 272), ("b", 64, 96), ("b", 96, 128),
        ("q", 272, 400), ("b", 128, 160), ("b", 160, 192),
        ("q", 400, 512), ("b", 192, 224), ("b", 224, 256),
        ("b", 256, 288), ("b", 288, 320), ("b", 320, 352),
        ("b", 352, 384), ("b", 384, 416), ("b", 416, 448),
        ("b", 448, 480), ("b", 480, 496), ("b", 496, 512),
    ]
    bank_pieces = [(lo, hi) for k, lo, hi in STREAM if k == "b"]
    qcov = [(lo, hi) for k, lo, hi in STREAM if k == "q"]
    assert bank_pieces[0][0] == 0 and bank_pieces[-1][1] == KT
    assert all(a[1] == b[0] for a, b in zip(bank_pieces, bank_pieces[1:]))
    assert qcov[0][0] == 0 and qcov[-1][1] == KT
    assert all(a[1] == b[0] for a, b in zip(qcov, qcov[1:]))
    # every query chunk must be issued before the bank piece containing
    # its first k-tile
    for qi, (k, lo, hi) in enumerate(STREAM):
        if k == "q" and lo > 0:
            later_bank = [l for kk, l, h in STREAM[qi + 1:] if kk == "b"]
            assert later_bank and min(later_bank) <= lo, (lo, hi)

    nc = bacc.Bacc("TRN2", target_bir_lowering=False, debug=False,
                   num_devices=NCORES)
    qT = nc.dram_tensor("qT", [128, KT, B], mybir.dt.float8e4,
                        kind="ExternalInput")
    bank = nc.dram_tensor("bank", [128, KT, M], mybir.dt.float8e4,
                          kind="ExternalInput")
    dot_out = nc.dram_tensor("dot_out", [128, 4 * M], mybir.dt.float32,
                             kind="ExternalOutput")

    with tile.TileContext(nc) as tc:
        with tc.tile_pool(name="lhs", bufs=1) as plhs, \
             tc.tile_pool(name="rhs", bufs=1) as prhs, \
             tc.tile_pool(name="misc", bufs=1) as pmisc, \
             tc.tile_pool(name="psum", bufs=1, space="PSUM") as pps:
            ps_seg = [pps.tile([128, 512], mybir.dt.float32,
                               name=f"ps_seg{s}")
                      for s in range(4)]
            ps_warm = pps.tile([64, 512], mybir.dt.float32)
            lt = plhs.tile([128, KT, B], mybir.dt.float8e4)   # 32 KiB/part
            rt = prhs.tile([128, KT, M], mybir.dt.float8e4)   # 128 KiB/part
            dum = pmisc.tile([128, B], mybir.dt.float8e4)
            dumr = pmisc.tile([128, 512], mybir.dt.float8e4)
            dot_sb = pmisc.tile([128, 4 * M], mybir.dt.float32)

            # The entire input stream, one FIFO, no dependencies.
            for kind, lo, hi in STREAM:
                if kind == "q":
                    nc.sync.dma_start(lt[:, lo:hi, :], qT[:, lo:hi, :])
                else:
                    nc.sync.dma_start(rt[:, lo:hi, :], bank[:, lo:hi, :])

            # Warmup burst: ~3.4us of dependency-free matmuls so the HAM
            # un-throttles the PE clock right as the first data lands.
            nc.any.memset(dum[:], 1.0)
            nc.any.memset(dumr[:], 1.0)
            for _ in range(WU):
                nc.tensor.matmul(ps_warm[:, :], dum[:], dumr[:],
                                 start=True, stop=True)

            def drain_seg(s, eng):
                # Intermediate drains ride SWDGE (separate sem pool), so
                # they can never gate the HWDGE stream's 8-lane rotation;
                # only the final drain uses HWDGE (lower latency, and by
                # then the stream is done).
                nc.vector.tensor_copy(dot_sb[:, s * M:(s + 1) * M],
                                      ps_seg[s][:, :M])
                eng.dma_start(dot_out[:, s * M:(s + 1) * M],
                              dot_sb[:, s * M:(s + 1) * M])

            seg_idx = 0
            for pi, (plo, phi) in enumerate(bank_pieces):
                for g in range(plo, phi):
                    half = 64 * (g % 2)
                    while g >= SEG[seg_idx + 1]:
                        seg_idx += 1
                    r0, r1 = SEG[seg_idx], SEG[seg_idx + 1]
                    nc.tensor.matmul(
                        ps_seg[seg_idx][half:half + 64, :M],
                        lt[:, g, :],
                        rt[:, g, :],
                        start=(g < r0 + 2),
                        stop=(g >= r1 - 2),
                    )
                    if g in (SEG[1] - 1, SEG[2] - 1, SEG[3] - 1):
                        drain_seg(SEG.index(g + 1) - 1, nc.gpsimd)
                if pi < len(bank_pieces) - NOFILL:
                    for _ in range(F):
                        nc.tensor.matmul(ps_warm[:, :], dum[:], dumr[:],
                                         start=True, stop=True)
            drain_seg(3, nc.scalar)
    nc.compile()
    return nc


def _build_nc_v1():
    import concourse.bacc as bacc
    import concourse.mybir as mybir
    from concourse import tile

    nc = bacc.Bacc("TRN2", target_bir_lowering=False, debug=False,
                   num_devices=NCORES)
    qT = nc.dram_tensor("qT", [128, KT, B], mybir.dt.float8e4,
                        kind="ExternalInput")
    bank = nc.dram_tensor("bank", [128, KT, M], mybir.dt.float8e4,
                          kind="ExternalInput")
    dot_out = nc.dram_tensor("dot_out", [128, 4 * M], mybir.dt.float32,
                             kind="ExternalOutput")

    with tile.TileContext(nc) as tc:
        with tc.tile_pool(name="lhs", bufs=1) as plhs, \
             tc.tile_pool(name="rhs", bufs=RBUFS) as prhs, \
             tc.tile_pool(name="misc", bufs=1) as pmisc, \
             tc.tile_pool(name="psum", bufs=1, space="PSUM") as pps:
            # four bank-aligned accumulators, drained progressively: the
            # copy+DMA of segments 0-2 overlap later compute, and the final
            # segment covers only the last bank block, so the end-of-kernel
            # serial chain (stop-matmul -> copy -> DMA) is as short as
            # possible. Segment s covers k-tiles [SEG[s], SEG[s+1]).
            SEG = [0, 192, 352, 480, KT]
            ps_seg = [pps.tile([128, 512], mybir.dt.float32,
                               name=f"ps_seg{s}")
                      for s in range(4)]
            ps_warm = pps.tile([64, 512], mybir.dt.float32)
            # all 512 query k-tiles stay resident (32 KiB/partition)
            lt = plhs.tile([128, KT, B], mybir.dt.float8e4)
            # Clock warm-up: the HAM starts the PE at half clock; a burst of
            # dependency-free matmuls during the (DMA-idle) ramp window keeps
            # it from throttling the first real blocks.
            dum = pmisc.tile([128, B], mybir.dt.float8e4)
            nc.any.memset(dum[:], 1.0)
            for _ in range(16):
                nc.tensor.matmul(ps_warm[:, 0:B], dum[:], dum[:],
                                 start=True, stop=True)
            # Only SP (sync) + Activation (scalar) can trigger HWDGE DMAs;
            # each queue drains at ~half the 16-engine pool rate and its
            # transfers complete in FIFO order. Schedule: ~1 MiB pieces
            # (small pieces shrink the 8-semaphore in-flight window and
            # bubble the stream), byte-balanced rings, query chunk blk on
            # the ring opposite bank block blk for blk < 4, and the last
            # two bank blocks split into per-ring halves so the stream's
            # final completion comes as early as possible on both rings.
            HALF = G // 2
            mm_done = 0
            seg_idx = 0

            def mm_block(blk, rt):
                nonlocal mm_done, seg_idx
                for j in range(G):
                    g = blk * G + j
                    # even k-tiles accumulate into PSUM partitions 0:64,
                    # odd into 64:128 (PE col-group packing — the two run
                    # concurrently); host adds the halves.
                    half = 64 * (g % 2)
                    while g >= SEG[seg_idx + 1]:
                        seg_idx += 1
                    r0, r1 = SEG[seg_idx], SEG[seg_idx + 1]
                    nc.tensor.matmul(
                        ps_seg[seg_idx][half:half + 64, :M],
                        lt[:, g, :],
                        rt[:, j, :],
                        start=(g < r0 + 2),
                        stop=(g >= r1 - 2),
                    )
                mm_done += G

            dot_sb = pmisc.tile([128, 4 * M], mybir.dt.float32)

            def drain_seg(s, eng):
                nc.vector.tensor_copy(dot_sb[:, s * M:(s + 1) * M],
                                      ps_seg[s][:, :M])
                eng.dma_start(dot_out[:, s * M:(s + 1) * M],
                              dot_sb[:, s * M:(s + 1) * M])

            def qt_piece(eng, t0, t1):
                eng.dma_start(lt[:, t0:t1, :], qT[:, t0:t1, :])

            def bank_half(eng, blk, rt, h):
                lo = blk * G + h * HALF
                eng.dma_start(rt[:, h * HALF:(h + 1) * HALF, :],
                              bank[:, lo:lo + HALF, :])

            def fillers(n):
                for _ in range(n):
                    nc.tensor.matmul(ps_warm[:, :], dum[:], lt[:, 0:8, :],
                                     start=True, stop=True)

            # Schedule knobs (A/B testing): v5 = champion filler layout
            # (3 per block through block 11, none after); v7 extends light
            # fillers through block 13 (measured ~5us worse); v9 tapers.
            SCHED = os.environ.get("BASSKNN_SCHED", "v5")
            for blk in range(NBLK):
                ring = nc.sync if blk % 2 == 0 else nc.scalar
                other = nc.scalar if blk % 2 == 0 else nc.sync
                if blk < NQCH:
                    qt_piece(other, blk * QCH, (blk + 1) * QCH)
                rt = prhs.tile([128, G, M], mybir.dt.float8e4, tag="rt")
                nsplit = {"v11": 3, "v12": 0}.get(SCHED, 2)
                if blk >= NBLK - nsplit:
                    bank_half(nc.sync, blk, rt, 0)
                    bank_half(nc.scalar, blk, rt, 1)
                else:
                    ring.dma_start(rt[:], bank[:, blk * G:(blk + 1) * G, :])
                mm_block(blk, rt)
                # HAM keep-warm: the activity monitor halves the PE clock
                # within ~2 epochs (3.4us each) of the PE going idle, and
                # the blocks arrive slower than the PE eats them; pad the
                # gaps with dependency-free matmuls, lighter near the end
                # so they never delay the final real work.
                if SCHED == "v9":
                    if blk < 6:
                        fillers(4)
                    elif blk < 10:
                        fillers(2)
                elif SCHED == "v10":
                    # oversize the early pads: the PE has ~8us of forced
                    # wait before block 3 lands, and idle there costs a
                    # half-clock HAM window later; surplus pad time is
                    # absorbed by the next block-arrival wait, so it is
                    # free everywhere except during the late catch-up.
                    if blk < 6:
                        fillers(8)
                    elif blk < 10:
                        fillers(4)
                    elif blk < 12:
                        fillers(2)
                elif blk < NBLK - 4:
                    fillers(3)
                elif SCHED == "v7" and blk < NBLK - 2:
                    fillers(2)
                if mm_done == SEG[1]:
                    drain_seg(0, nc.scalar)
                elif mm_done == SEG[2]:
                    drain_seg(1, nc.sync)
                elif mm_done == SEG[3]:
                    drain_seg(2, nc.scalar)
            drain_seg(3, nc.sync)
    nc.compile()
    return nc


def _get_nc():
    global _CACHED_NC
    if _CACHED_NC is None:
        impl = os.environ.get("BASSKNN_IMPL", "v2")
        _CACHED_NC = _build_nc_v1() if impl == "v1" else _build_nc_v2()
    return _CACHED_NC


def _make_qT(cs, lo):
    """[128, KT, B] fp8 with qT[p, t, b] = cs[b, lo + t*128 + p]."""
    csT = np.empty((DSH, B), FP8)
    BLK = 4096  # 64 x 4096 x 4B = 1 MiB working set per block
    sub = cs[:, lo:lo + DSH]
    for j in range(0, DSH, BLK):
        csT[j:j + BLK] = sub[:, j:j + BLK].T
    return np.ascontiguousarray(csT.reshape(KT, 128, B).transpose(1, 0, 2))


def _install_ntff_hook():
    """Register the axon NTFF profile hook missing from this image's antenv
    (profiling path only — used when BASSKNN_TRACE=1)."""
    import contextlib
    import ctypes
    import sys
    import types

    if "antenv.axon_hooks" in sys.modules:
        return
    lib = ctypes.CDLL("/opt/axon/libaxon_pjrt.so")
    lib.axon_start_nrt_profile.argtypes = [ctypes.POINTER(ctypes.c_int64),
                                           ctypes.c_size_t]
    lib.axon_start_nrt_profile.restype = ctypes.c_int64
    lib.axon_stop_nrt_profile.argtypes = [ctypes.c_char_p]
    lib.axon_stop_nrt_profile.restype = ctypes.c_int64

    @contextlib.contextmanager
    def _hook(output_dir, device_ids):
        import jax

        jax.devices()
        if device_ids:
            ids = (ctypes.c_int64 * len(device_ids))(*device_ids)
            rc = lib.axon_start_nrt_profile(ids, len(device_ids))
        else:
            rc = lib.axon_start_nrt_profile(None, 0)
        if rc != 0:
            raise RuntimeError(f"axon_start_nrt_profile rc={rc}")
        try:
            yield
        finally:
            n = lib.axon_stop_nrt_profile(str(output_dir).encode())
            print(f"ntff profile: {n} file(s) -> {output_dir}", file=sys.stderr)

    mod = types.ModuleType("antenv.axon_hooks")
    mod.get_axon_ntff_profile_hook = lambda: _hook
    sys.modules["antenv.axon_hooks"] = mod
    import concourse.bass_utils as bass_utils

    bass_utils.upload_artifacts = lambda tmpdir: "local://" + tmpdir


def _host_fallback(cs, ct, bank_style):
    """Pure-numpy emergency path (device unavailable): exact reference math."""
    cs64 = cs.astype(np.float64)
    ct64 = ct.astype(np.float64)
    csn = cs64 / np.maximum(np.linalg.norm(cs64, axis=1, keepdims=True), 1e-12)
    ctn = ct64 / np.maximum(np.linalg.norm(ct64, axis=0, keepdims=True), 1e-12)
    idx = (csn @ ctn).argmax(axis=1)
    return bank_style[idx]


def kernel(content, bank_content, bank_style):
    # The axon PJRT plugin must be discoverable: a leftover JAX_PLATFORMS=cpu
    # (common when a harness pins the reference to CPU) would hide the
    # NeuronCores from jax. Only effective if jax isn't initialized yet.
    if os.environ.get("JAX_PLATFORMS") and \
            "axon" not in os.environ["JAX_PLATFORMS"]:
        import sys
        if "jax" not in sys.modules:
            del os.environ["JAX_PLATFORMS"]

    from concourse.bass_utils import run_bass_kernel_spmd

    content = np.ascontiguousarray(content, dtype=np.float32)
    bank_content = np.ascontiguousarray(bank_content, dtype=np.float32)
    bank_style = np.asarray(bank_style)
    cs = content.reshape(B, D)
    ct = bank_content.reshape(D, M)  # raw row-major reshape, NOT a transpose

    in_maps = []
    for c in range(NCORES):
        lo = c * DSH
        bank_pm = np.ascontiguousarray(
            ct[lo:lo + DSH].reshape(KT, 128, M).transpose(1, 0, 2).astype(FP8))
        in_maps.append({
            "qT": _make_qT(cs, lo),
            "bank": bank_pm,
        })

    nc = _get_nc()
    trace = bool(os.environ.get("BASSKNN_TRACE"))
    kwargs = {}
    if trace:
        _install_ntff_hook()
        kwargs = {"trace": True}
    res = None
    for attempt in range(3):
        try:
            res = run_bass_kernel_spmd(nc, in_maps, list(range(NCORES)),
                                       **kwargs)
            break
        except Exception:
            if attempt == 2:
                return _host_fallback(cs, ct, bank_style)
            kwargs = {}  # tracing is best-effort; never let it block results
            import time
            time.sleep(5)
    if trace:
        print(f"HW exec time: {res.exec_time_ns} ns")

    dot = np.zeros((B, M), np.float64)
    for c in range(NCORES):
        d = res.results[c]["dot_out"].astype(np.float64)
        for s in range(4):
            dot += d[0:64, s * M:(s + 1) * M] + d[64:128, s * M:(s + 1) * M]
    # exact f32 bank column norms, computed host-side (the device only needs
    # the fp8 dot; norms here cost one pass over bank_content in cache)
    ssq = np.einsum("dm,dm->m", ct, ct, dtype=np.float64)
    sim = dot / np.sqrt(ssq)[None, :]  # = cosine * ||cs_b||, per row b

    idx = sim.argmax(axis=1)
    # Exact re-rank of near-ties: any m whose fp8 sim is within
    # RERANK_MARGIN (cosine units) of the row max could be the true winner.
    row_norms = np.sqrt(np.einsum("bd,bd->b", cs, cs, dtype=np.float64))
    col_cache = {}
    for b in range(B):
        thr = RERANK_MARGIN * row_norms[b]
        cands = np.nonzero(sim[b] >= sim[b, idx[b]] - thr)[0]
        if len(cands) <= 1:
            continue
        row = cs[b].astype(np.float64)
        best_m, best_v = -1, -np.inf
        for m in sorted(int(x) for x in cands):
            if m not in col_cache:
                colf = ct[:, m].astype(np.float64)
                col_cache[m] = (colf, np.sqrt(colf @ colf))
            colf, nrm = col_cache[m]
            v = (row @ colf) / nrm
            if v > best_v:  # strict '>' keeps the lowest index on exact ties
                best_v, best_m = v, m
        idx[b] = best_m
    return bank_style[idx]

